# revision 28
# baseline (speedup 1.0000x reference)
"""Lensiformer forward pass on 8 Trainium2 NeuronCores.

Strategy: data-parallel over batch (32 images -> 4 per core, params
replicated, no collectives). Per core, a single fused Bass/Tile program
runs the whole network.

v3 (attention head-pair rework of the v2 engine-balance build):
  - bf16 matmul operands everywhere in the transformer (residual stream,
    LN statistics and PSUM accumulation stay fp32)
  - attention processed as head PAIRS (2j, 2j+1): the pair's score and
    tail matmuls sit at partition halves 0:64 / 64:128 of the shared fc
    columns, so they auto-derive distinct PE row groups and run
    concurrently when issued adjacently
  - no mask matmuls: the self-attention diagonal is zeroed post-exp on
    SBUF bf16 (pt *= 1-I); the per-image tail key (token 256) gets a
    1-element memset; NQ=257 (no pad column)
  - tail-token V for all 4 images via one strided-lhsT matmul set per
    layer, staged to SBUF and DMA-scattered to partitions 0/32 so the
    pair's pv-tail matmuls land on distinct row groups
  - softmax denominator via a 65th all-ones V column; 1/denom =
    exp(-ln(d)) with the pair's lns at partitions 0/32 and one batched
    exp; rank-1 broadcasts into the pv dead rows (concurrent pair)
  - software-pipelined emission: stage1(unit k) scores/exps | stage2
    (unit k-1) pv/denoms | one dense proj/B_v filler per slot | stage2b
    (unit k-1) broadcast+ot-apply, so the PE FIFO rarely waits on the
    cross-engine softmax chain
  - LN applies on DVE (tensor_scalar) instead of ACT; MLP1 tail tokens
    batched into one PSUM bank with host-prepared b1tail bias and a
    single gelu
  - LN rstd = exp(-0.5*ln(var+eps)) so LN + attention exp + copies all
    live in one ACT table; gelu is the only table switch (2/layer)
  - PSUM: 4 x 1-bank rotating tiles + 2 x 2-bank score tiles

Self-contained: includes the walrus sync-wait-limit workaround and the
axon NTFF profiling shim.
"""
import contextlib
import ctypes
import sys
import types

import numpy as np
import ml_dtypes

import concourse.bass as bass
import concourse.mybir as mybir
import concourse.tile as tile
from concourse.masks import make_identity
from concourse.vector_clock import ScopedClock

F32 = mybir.dt.float32
F32R = mybir.dt.float32r
BF16 = mybir.dt.bfloat16
AF = mybir.ActivationFunctionType
ALU = mybir.AluOpType

# ---------------- model geometry (hardcoded from the problem spec) ----------
B, IMG, PATCH = 32, 128, 8
D, H, L, MLP, NCLS = 512, 8, 8, 2048, 3
GRID = IMG // PATCH            # 16
P = GRID * GRID                # 256 patches / image
N = P + 1                      # 257 tokens / image
HD = D // H                    # 64
KC = 320                       # im2col contraction: 5 shifts * 8 * 8
NCORES = 8
NIMG = B // NCORES             # 4 images / core
TP = NIMG * P                  # 1024 patch tokens / core
NT = NIMG * N                  # 1028 transformer tokens / core
NTILE = 9                      # token tiles of 128
TT = NTILE * 128               # 1152 padded tokens
IMGOFF = [i * N for i in range(NIMG)]
NEG = -1.0e30
NQ = 257                       # score q width (real q cols only)
NQR = 257                      # real q cols
QKW = NT                       # qkt cols
BF_NP = ml_dtypes.bfloat16

_PROGRAM_CACHE = {}

# ============================================================================
# environment fixups
# ============================================================================
_fixups_done = False


def _install_fixups():
    global _fixups_done
    if _fixups_done:
        return
    _fixups_done = True
    MAXW = 1

    def _split_waits(nc, ordered):
        for bb_name, insts in ordered.items():
            new_list = []
            for inst in insts:
                si = getattr(inst, 'sync_info', None)
                eng = getattr(inst, 'engine', None)
                if (si is not None and si.on_wait and len(si.on_wait) > MAXW
                        and eng is not None
                        and type(inst).__name__.startswith('Inst')):
                    waits = list(si.on_wait)
                    inst.sync_info = mybir.SyncInfo(
                        on_wait=waits[:MAXW], on_update=list(si.on_update or []))
                    for i in range(MAXW, len(waits), MAXW):
                        new_list.append(mybir.InstNoOp(
                            name=nc.get_next_instruction_name(),
                            engine=eng, bass_nofuse=True,
                            sync_info=mybir.SyncInfo(
                                on_wait=waits[i:i + MAXW], on_update=[])))
                new_list.append(inst)
            ordered[bb_name] = new_list

    orig_lower = tile.TileContext._lower_ordered_insts

    def patched_lower(self, ordered):
        _split_waits(self.nc, ordered)
        return orig_lower(self, ordered)

    tile.TileContext._lower_ordered_insts = patched_lower

    def patched_drain_and_barrier(self, tick_clock, wait_clock):
        drain_inst = self.nc.sync.drain()
        wait_clock.add_sem_waits(
            drain_inst.ins, ScopedClock({None: tick_clock.global_clock}))
        si = drain_inst.ins.sync_info
        if si and si.on_wait and len(si.on_wait) > MAXW:
            waits = list(si.on_wait)
            drain_inst.ins.sync_info = mybir.SyncInfo(
                on_wait=waits[:MAXW], on_update=list(si.on_update or []))
            for i in range(MAXW, len(waits), MAXW):
                extra = self.nc.sync.drain()
                extra.ins.sync_info = mybir.SyncInfo(
                    on_wait=waits[i:i + MAXW], on_update=[])
        self.nc.all_engine_barrier()
        assert self.sems is not None
        popped = self.nc._tile_sem_poison_stack.pop()
        assert popped is self._sem_poison
        self.nc.clear_and_free_semaphores(list(self.sems.allocated().values()))
        self.nc.all_engine_barrier()

    tile.TileContext._drain_and_barrier = patched_drain_and_barrier

    if 'antenv.axon_hooks' not in sys.modules:
        holder = {'h': None}
        mod = types.ModuleType('antenv.axon_hooks')
        mod.set_axon_ntff_profile_hook = lambda h: holder.__setitem__('h', h)
        mod.get_axon_ntff_profile_hook = lambda: holder['h']
        sys.modules['antenv.axon_hooks'] = mod
        try:
            lib = ctypes.CDLL('/opt/axon/libaxon_pjrt.so')
            if hasattr(lib, 'axon_start_nrt_profile'):
                lib.axon_start_nrt_profile.argtypes = [
                    ctypes.POINTER(ctypes.c_int64), ctypes.c_size_t]
                lib.axon_start_nrt_profile.restype = ctypes.c_int64
                lib.axon_stop_nrt_profile.argtypes = [ctypes.c_char_p]
                lib.axon_stop_nrt_profile.restype = ctypes.c_int64

                @contextlib.contextmanager
                def _hook(output_dir, device_ids):
                    import jax
                    jax.devices()
                    if device_ids:
                        ids = (ctypes.c_int64 * len(device_ids))(*device_ids)
                        rc = lib.axon_start_nrt_profile(ids, len(device_ids))
                    else:
                        rc = lib.axon_start_nrt_profile(None, 0)
                    if rc != 0:
                        raise RuntimeError(f'axon_start_nrt_profile rc={rc}')
                    try:
                        yield
                    finally:
                        lib.axon_stop_nrt_profile(output_dir.encode())

                mod.set_axon_ntff_profile_hook(_hook)
        except OSError:
            pass


# ============================================================================
# host-side input marshaling (pure data movement + tiny param folds)
# ============================================================================
def _im2col(image):
    """(Bc,1,IMG,IMG) -> (Bc, P, 320), col order [shift, py, px]."""
    shifts = [(0, 0), (1, 1), (-1, 1), (1, -1), (-1, -1)]
    x = image[:, 0]
    cols = []
    for (sy, sx) in shifts:
        xs = np.roll(x, (sy, sx), (1, 2))
        pt = xs.reshape(-1, GRID, PATCH, GRID, PATCH).transpose(0, 1, 3, 2, 4)
        cols.append(pt.reshape(-1, P, PATCH * PATCH))
    return np.concatenate(cols, -1)


def _bf(a):
    return np.ascontiguousarray(np.asarray(a, np.float32).astype(BF_NP))


def _host_prep(inputs):
    f = lambda k: np.ascontiguousarray(np.asarray(inputs[k], np.float32))
    image = f('image')

    # conv weights -> matmul form, both tokenizers side by side
    wconv = np.concatenate(
        [f('ssw').reshape(D, KC).T, f('sow').reshape(D, KC).T], 1)  # (320,1024)
    bconv = np.concatenate([f('ssb'), f('sob')])                    # (1024,)
    gbeta = np.stack([np.concatenate([f('ssg'), f('sog')]),
                      np.concatenate([f('ssbeta'), f('sobeta')])])  # (2,1024)

    # fold LN gains/biases into the following matmuls (exact rewrite)
    ln1g, ln1b = f('ln1g'), f('ln1b')
    ln2g, ln2b = f('ln2g'), f('ln2b')
    qkvw, qkvb = f('qkvw'), f('qkvb')
    projw, projb = f('projw'), f('projb')
    w1, b1 = f('w1'), f('b1')
    qkvw_eff = ln1g[:, :, None] * qkvw
    qkvb_eff = qkvb + np.einsum('ld,ldn->ln', ln1b, qkvw)
    w1_eff = ln2g[:, :, None] * w1
    b1_eff = b1 + np.einsum('ld,ldn->ln', ln2b, w1)
    hw_eff = f('ng')[:, None] * f('hw')
    # V bias contributes exactly bv @ projw to the proj output (softmax
    # weights sum to 1 after the 1/denom divide) -> fold into projb
    bv = qkvb_eff[:, 2 * D:3 * D]                      # (L, 512)
    projb_eff = projb + np.einsum('ld,ldn->ln', bv, projw)

    # pos/cls in padded transformer layout
    pos = f('pos_embed')[0]          # (257, 512)
    cls_eff = f('cls_token')[0, 0] + pos[0]
    pospad = np.zeros((TT, D), np.float32)
    for i in range(NIMG):
        pospad[IMGOFF[i]] = cls_eff
        pospad[IMGOFF[i] + 1: IMGOFF[i] + N] = pos[1:]

    X = _im2col(image)               # (B, P, 320)

    # MLP1 tail bias, pre-broadcast for the batched tail bank:
    # b1tail[l, p, hc*4 + j] = b1_eff[l, hc*128 + p]
    b1tail = np.repeat(
        b1_eff.reshape(L, MLP // 128, 128).transpose(0, 2, 1), 4, axis=2)

    common = dict(
        wconv=_bf(wconv), bconv=_bf(bconv), gbeta=_bf(gbeta),
        fw=_bf(f('fw')), fb=_bf(f('fb')), pospad=pospad,
        qkvw=_bf(qkvw_eff),
        qkvbqk=np.ascontiguousarray(qkvb_eff[:, 0:2 * D]),
        projw=_bf(projw), projb=_bf(projb_eff), temp=f('temp'),
        w1=_bf(w1_eff), b1=np.ascontiguousarray(b1_eff),
        b1tail=np.ascontiguousarray(b1tail),
        w2=_bf(f('w2')), b2=_bf(f('b2')),
        hw=np.ascontiguousarray(
            np.concatenate([hw_eff, np.zeros((D, 1), np.float32)], 1)),
        hb=np.ascontiguousarray(
            np.concatenate([f('hb') + f('nb') @ f('hw'),
                            np.zeros(1, np.float32)])),
    )
    in_maps = []
    for c in range(NCORES):
        # token-layout im2col: col = transformer token index, cls cols zero
        xt = np.zeros((KC, NT), np.float32)
        for i in range(NIMG):
            xt[:, IMGOFF[i] + 1:IMGOFF[i] + N] = X[c * NIMG + i].T
        m = dict(common)
        m['xt'] = _bf(xt)
        in_maps.append(m)
    return in_maps


# ============================================================================
# device program
# ============================================================================
def _tile_segments(t):
    """Real-token segments of token-tile t: (row_in_tile, n, img, pos0)."""
    segs = []
    r0 = 128 * t
    for img in range(NIMG):
        lo = max(r0, IMGOFF[img])
        hi = min(r0 + 128, IMGOFF[img] + N, NT)
        if lo < hi:
            segs.append((lo - r0, hi - lo, img, lo - IMGOFF[img]))
    return segs


def _tile_rows(t):
    """Number of real token rows in token-tile t."""
    return min(128, max(0, NT - 128 * t))


def _build_program():
    nc = bass.Bass()

    din = lambda nm, sh, dt_=F32: nc.dram_tensor(nm, sh, dt_, kind='ExternalInput')
    xt_d = din('xt', [KC, NT], BF16)
    wc_d = din('wconv', [KC, 2 * D], BF16)
    bc_d = din('bconv', [2 * D], BF16)
    gb_d = din('gbeta', [2, 2 * D], BF16)
    fw_d = din('fw', [2 * D, D], BF16)
    fb_d = din('fb', [D], BF16)
    pos_d = din('pospad', [TT, D])
    qkvw_d = din('qkvw', [L, D, 3 * D], BF16)
    qkvbqk_d = din('qkvbqk', [L, 2 * D])
    projw_d = din('projw', [L, D, D], BF16)
    projb_d = din('projb', [L, D], BF16)
    temp_d = din('temp', [L, H])
    w1_d = din('w1', [L, D, MLP], BF16)
    b1_d = din('b1', [L, MLP])
    b1t_d = din('b1tail', [L, 128, 64])
    w2_d = din('w2', [L, MLP, D], BF16)
    b2_d = din('b2', [L, D], BF16)
    hw_d = din('hw', [D, 4])
    hb_d = din('hb', [4])
    out_d = nc.dram_tensor('out', [NIMG, 4], F32, kind='ExternalOutput')

    with tile.TileContext(nc) as tc, \
            nc.allow_low_precision(reason='bf16 matmul operands'):
        with contextlib.ExitStack() as ctx:
            sb = ctx.enter_context(tc.tile_pool(name='sb', bufs=1))
            ps = ctx.enter_context(tc.tile_pool(name='ps', bufs=4, space='PSUM'))
            ps2 = ctx.enter_context(tc.tile_pool(name='ps2', bufs=2, space='PSUM'))

            _psn = [0]

            def pstile(dt_=F32):
                _psn[0] += 1
                return ps.tile([128, 512], dt_, tag='ps', bufs=4,
                               name=f'ps{_psn[0]}')

            # ---------------- constants ----------------
            ident = sb.tile([128, 128], F32, tag='ident')
            make_identity(nc, ident)
            ident_bf = sb.tile([128, 128], BF16, tag='identbf')
            nc.vector.tensor_copy(out=ident_bf, in_=ident)
            # 1 - I, to zero the self-attention diagonal post-exp
            om_id = sb.tile([128, 128], BF16, tag='omid')
            nc.vector.memset(om_id, 1.0)
            nc.vector.tensor_sub(om_id, om_id, ident_bf)
            # all-ones block whose rows 0 / 32 serve as 1/denom broadcast
            # lhsT for the pair (distinct row groups -> concurrent matmuls)
            onesb = sb.tile([64, 64], BF16, tag='onesb')
            nc.vector.memset(onesb, 1.0)
            ones_row = sb.tile([1, 128], BF16, tag='ones_row')
            nc.vector.memset(ones_row, 1.0)
            eps = sb.tile([128, 1], F32, tag='eps')
            nc.vector.memset(eps, 1e-5)

            # ---------------- persistent activations ----------------
            tok = sb.tile([128, NTILE, D], F32, tag='tok')       # residual

            lay = None  # transformer pool; opened after the embed pool closes

            # ---- LN helpers (split so ACT parts can be emission-grouped) ---
            def ln_stats(t):
                stats = lay.tile([128, 6], F32, tag='lnstat', bufs=10)
                mv = lay.tile([128, 2], F32, tag='lnmv', bufs=10)
                nc.vector.bn_stats(out=stats, in_=tok[:, t, :])
                nc.vector.bn_aggr(out=mv, in_=stats)
                return mv

            def ln_rstd(mv, fence=None):
                """Ln/Exp rstd chain (the ACT-table-sensitive part). `fence`
                delays it (via a value-preserving 1-element rewrite of mv)
                so the scheduler cannot pull the Ln/Exp into a gelu stretch
                and thrash the ACT table."""
                if fence is not None:
                    nc.vector.scalar_tensor_tensor(
                        out=mv[0:1, :], in0=mv[0:1, :], scalar=1.0,
                        in1=fence, op0=ALU.mult, op1=ALU.bypass)
                lnv = lay.tile([128, 1], F32, tag='lnv', bufs=10)
                nc.scalar.activation(out=lnv, in_=mv[:, 1:2], func=AF.Ln,
                                     bias=eps, scale=1.0)
                rstd = lay.tile([128, 1], F32, tag='lnrstd', bufs=10)
                nc.scalar.activation(out=rstd, in_=lnv, func=AF.Exp,
                                     scale=-0.5)
                nmr = lay.tile([128, 1], F32, tag='lnnmr', bufs=10)
                nc.vector.scalar_tensor_tensor(
                    out=nmr, in0=mv[:, 0:1], scalar=-1.0,
                    in1=rstd, op0=ALU.mult, op1=ALU.mult)
                return rstd, nmr

            def ln_apply_tr(t, rstd, nmr, xT_dst):
                """LN apply on DVE (frees ACT) + transposes + copy."""
                xn = lay.tile([128, D], BF16, tag='xn', bufs=3)
                nc.vector.tensor_scalar(
                    out=xn, in0=tok[:, t, :], scalar1=rstd, scalar2=nmr,
                    op0=ALU.mult, op1=ALU.add)
                tp = pstile(BF16)
                for c in range(4):
                    nc.tensor.transpose(tp[:, c * 128:(c + 1) * 128],
                                        xn[:, c * 128:(c + 1) * 128], ident_bf)
                nc.vector.tensor_copy(
                    out=xT_dst[:, :, t * 128:(t + 1) * 128], in_=tp)

            def ln_stats_rstd(t, fence=None):
                return ln_rstd(ln_stats(t), fence=fence)

            # tile groups matching the 512/512/tail column groups that
            # B_qk / MLP1 consume; applies interleave with those groups
            _LN_GROUPS = ((0, 1, 2, 3), (4, 5, 6, 7), (8,))

            def ln_sweep(rn, xT_dst, group_emitters=None):
                for gi, tiles in enumerate(_LN_GROUPS):
                    for t in tiles:
                        ln_apply_tr(t, rn[t][0], rn[t][1], xT_dst)
                    if group_emitters is not None:
                        group_emitters[gi]()

            # ================= patch embed =================
            # the im2col input is already in transformer token layout (cls
            # cols zero), so conv/gate/fuse run per token tile and tokens
            # land in `tok` without a DRAM reshuffle
            with tc.tile_pool(name='emb', bufs=1) as emb:
                xt_sb = []
                for kc, k0, kn in ((0, 0, 128), (1, 128, 128), (2, 256, 64)):
                    t_ = emb.tile([kn, NT], BF16, tag=f'xt{kc}')
                    nc.sync.dma_start(out=t_, in_=xt_d[k0:k0 + kn, :])
                    xt_sb.append(t_)
                wc_sb = []
                for kc, k0, kn in ((0, 0, 128), (1, 128, 128), (2, 256, 64)):
                    t_ = emb.tile([kn, 2 * D], BF16, tag=f'wc{kc}')
                    nc.sync.dma_start(out=t_, in_=wc_d[k0:k0 + kn, :])
                    wc_sb.append(t_)
                bc_bc = emb.tile([128, 2 * D], BF16, tag='bc')
                nc.sync.dma_start(
                    out=bc_bc, in_=bc_d[None, :].to_broadcast([128, 2 * D]))
                gb_g = emb.tile([128, 2 * D], BF16, tag='gbg')
                nc.sync.dma_start(
                    out=gb_g, in_=gb_d[0][None, :].to_broadcast([128, 2 * D]))
                gb_b = emb.tile([128, 2 * D], BF16, tag='gbb')
                nc.sync.dma_start(
                    out=gb_b, in_=gb_d[1][None, :].to_broadcast([128, 2 * D]))
                fw_sb = emb.tile([128, 8, D], BF16, tag='fwsb')
                nc.sync.dma_start(
                    out=fw_sb, in_=fw_d[:, :].rearrange('(c p) n -> p c n', p=128))
                fb_bc = emb.tile([128, D], BF16, tag='fbsb')
                nc.sync.dma_start(out=fb_bc,
                                  in_=fb_d[None, :].to_broadcast([128, D]))

                # cls token rows per token tile: (tile, row)
                cls_rows = {IMGOFF[i] // 128: IMGOFF[i] % 128
                            for i in range(NIMG)}
                nc.vector.memset(tok[:, NTILE - 1, :], 0.0)
                combs, ggs = [], []
                # sweep 1: conv + LN + gains + transposes + gate matmul
                # (ACT stays in the ln/exp/identity table)
                for t in range(NTILE):
                    rows = _tile_rows(t)
                    c0 = t * 128
                    combraw = emb.tile([128, 2 * D], F32, tag='combraw', bufs=3)
                    for nh in range(2):
                        cps = pstile()
                        for kc in range(3):
                            nc.tensor.matmul(
                                cps[0:rows, :], xt_sb[kc][:, c0:c0 + rows],
                                wc_sb[kc][:, nh * D:(nh + 1) * D],
                                start=(kc == 0), stop=(kc == 2),
                                skip_group_check=True)
                        nc.vector.tensor_add(
                            combraw[0:rows, nh * D:(nh + 1) * D],
                            cps[0:rows, :],
                            bc_bc[0:rows, nh * D:(nh + 1) * D])

                    comb = emb.tile([128, 2 * D], BF16, tag='comb', bufs=9)
                    for nh in range(2):
                        sl = slice(nh * D, (nh + 1) * D)
                        stats = emb.tile([128, 6], F32, tag='estat', bufs=4)
                        mv = emb.tile([128, 2], F32, tag='emv', bufs=4)
                        nc.vector.bn_stats(out=stats[0:rows],
                                           in_=combraw[0:rows, sl])
                        nc.vector.bn_aggr(out=mv[0:rows], in_=stats[0:rows])
                        lnv = emb.tile([128, 1], F32, tag='elnv', bufs=4)
                        nc.scalar.activation(out=lnv[0:rows],
                                             in_=mv[0:rows, 1:2],
                                             func=AF.Ln, bias=eps[0:rows],
                                             scale=1.0)
                        rstd = emb.tile([128, 1], F32, tag='erstd', bufs=4)
                        nc.scalar.activation(out=rstd[0:rows], in_=lnv[0:rows],
                                             func=AF.Exp, scale=-0.5)
                        nmr = emb.tile([128, 1], F32, tag='enmr', bufs=4)
                        nc.vector.scalar_tensor_tensor(
                            out=nmr[0:rows], in0=mv[0:rows, 0:1], scalar=-1.0,
                            in1=rstd[0:rows], op0=ALU.mult, op1=ALU.mult)
                        nc.scalar.activation(out=comb[0:rows, sl],
                                             in_=combraw[0:rows, sl],
                                             func=AF.Identity,
                                             scale=rstd[0:rows],
                                             bias=nmr[0:rows])
                        last_rstd = rstd
                    nc.vector.tensor_mul(comb[0:rows], comb[0:rows], gb_g[0:rows])
                    nc.vector.tensor_add(comb[0:rows], comb[0:rows], gb_b[0:rows])

                    combT = emb.tile([128, 8, 128], BF16, tag='combT', bufs=3)
                    for half in range(2):
                        tp = ps2.tile([128, 2, 512], BF16, tag='stp2', bufs=2,
                                      name=f'etp{t}_{half}')
                        for c in range(4):
                            cc = half * 4 + c
                            nc.tensor.transpose(
                                tp[:, 0, c * rows:(c + 1) * rows],
                                comb[0:rows, cc * 128:(cc + 1) * 128],
                                ident_bf[0:rows, 0:rows])
                        nc.vector.tensor_copy(
                            out=combT[:, half * 4:(half + 1) * 4, 0:rows],
                            in_=tp[:, 0, 0:4 * rows].rearrange(
                                'p (c n) -> p c n', c=4))

                    gps = pstile()
                    for c in range(8):
                        nc.tensor.matmul(gps[0:rows, :], combT[:, c, 0:rows],
                                         fw_sb[:, c, :],
                                         start=(c == 0), stop=(c == 7),
                                         skip_group_check=True)
                    gg = emb.tile([128, D], F32, tag='gg', bufs=9)
                    nc.vector.tensor_add(gg[0:rows], gps[0:rows, :],
                                         fb_bc[0:rows, :])
                    combs.append(comb)
                    ggs.append(gg)

                # sweep 2: sigmoids, fenced behind the last embed-LN rstd
                # so the scheduler cannot interleave them with the Ln/Exp
                # chains and thrash the ACT table; then fuse into tok
                for t in range(NTILE):
                    rows = _tile_rows(t)
                    nc.vector.scalar_tensor_tensor(
                        out=ggs[t][0:1, 0:1], in0=ggs[t][0:1, 0:1],
                        scalar=1.0, in1=last_rstd[0:1, 0:1],
                        op0=ALU.mult, op1=ALU.bypass)
                    gt = emb.tile([128, D], F32, tag='gt', bufs=2)
                    nc.scalar.activation(out=gt[0:rows], in_=ggs[t][0:rows],
                                         func=AF.Sigmoid)
                    diff = emb.tile([128, D], F32, tag='diff', bufs=2)
                    nc.vector.tensor_sub(diff[0:rows], combs[t][0:rows, 0:D],
                                         combs[t][0:rows, D:2 * D])
                    nc.vector.tensor_mul(diff[0:rows], diff[0:rows],
                                         gt[0:rows])
                    nc.vector.tensor_add(diff[0:rows], diff[0:rows],
                                         combs[t][0:rows, D:2 * D])
                    postile = emb.tile([128, D], F32, tag='pos', bufs=3)
                    nc.gpsimd.dma_start(out=postile,
                                        in_=pos_d[t * 128:(t + 1) * 128, :])
                    nc.vector.tensor_add(tok[0:rows, t, :], diff[0:rows],
                                         postile[0:rows])
                    if t in cls_rows:
                        # cls rows carry conv-of-zeros junk; overwrite with
                        # cls_eff straight from pospad (DMA may address any
                        # partition, unlike the compute engines)
                        r = cls_rows[t]
                        nc.sync.dma_start(
                            out=tok[r:r + 1, t, :],
                            in_=pos_d[t * 128 + r:t * 128 + r + 1, :])

            # ================= transformer layers =================
            lay = ctx.enter_context(tc.tile_pool(name='lay', bufs=1))

            # after C(img), ot cols for these token tiles are complete
            _D_TILES = {0: (0, 1), 1: (2, 3), 2: (4, 5), 3: (6, 7, 8)}

            _BQK_GROUPS = ((0, 512), (512, 512), (1024, QKW - 1024))

            def bqk_group(qkvw_sb, qkvb_sb, qkt, xT, gi):
                g0, gw = _BQK_GROUPS[gi]

                def emit():
                    for fc in range(8):
                        qp = pstile()
                        for c in range(4):
                            nc.tensor.matmul(
                                qp[:, 0:gw],
                                qkvw_sb[:, c, fc * 128:(fc + 1) * 128],
                                xT[:, c, g0:g0 + gw],
                                start=(c == 0), stop=(c == 3))
                        nc.vector.tensor_scalar_add(
                            qkt[:, fc, g0:g0 + gw], qp[:, 0:gw],
                            qkvb_sb[:, fc:fc + 1])
                return emit

            def attn_proj_phase(l, xT, qkt, qkvw_sb, fillers=None):
                """B_v + C + proj(+residual) for layer l, head-PAIR units.

                Heads 2j/2j+1 share fc columns at partition halves 0:64 /
                64:128, so their score and tail matmuls auto-derive distinct
                PE row groups and run CONCURRENTLY when issued adjacently.
                The diagonal mask is applied post-exp on SBUF bf16 (pt *=
                1-I) instead of PSUM adds; both tail-score rows share one pv
                bank so a single ACT exp covers them; 1/denom is broadcast
                for the pair with one e2 matmul into the recycled score
                bank. Returns LN2 rstd chains per tile."""
                temp_sb = lay.tile([128, H], F32, tag='temp', bufs=2)
                nc.sync.dma_start(out=temp_sb,
                                  in_=temp_d[l][None, :].to_broadcast([128, H]))
                # per-pair tail-exp scale: temp[2j] at partition 32,
                # temp[2j+1] at partition 64 (one AP serves the batched exp)
                tpair = lay.tile([128, 4], F32, tag='tpair', bufs=2)
                tview = temp_sb.rearrange('p (j two) -> p j two', two=2)
                nc.vector.tensor_copy(out=tpair[0:1, :],
                                      in_=tview[0:1, :, 0])
                nc.vector.tensor_copy(out=tpair[32:33, :],
                                      in_=tview[32:33, :, 1])
                projw_sb = lay.tile([128, 4, D], BF16, tag='wt', bufs=3)
                nc.sync.dma_start(
                    out=projw_sb,
                    in_=projw_d[l].rearrange('(c p) n -> p c n', p=128))
                projb_bc = lay.tile([128, D], BF16, tag='projb', bufs=2)
                nc.sync.dma_start(out=projb_bc,
                                  in_=projb_d[l][None, :].to_broadcast([128, D]))

                vimg = lay.tile([128, NIMG, 2, H * 65], BF16, tag='vimg',
                                bufs=1)
                nc.gpsimd.memset(
                    vimg.rearrange('p i c (h e) -> p i c h e', e=65)
                    [:, :, :, :, 64:65], 1.0)
                # tail-token V rows, duplicated at partitions 32 (even head)
                # and 64 (odd head) so the pair's pv-tail matmuls land on
                # distinct row groups and run concurrently
                vl = lay.tile([128, NIMG, H * 65], BF16, tag='vlast', bufs=1)
                nc.gpsimd.memset(vl[0:1], 0.0)
                nc.gpsimd.memset(vl[32:33], 0.0)
                nc.gpsimd.memset(
                    vl.rearrange('p i (h e) -> p i h e', e=65)
                    [0:1, :, :, 64:65], 1.0)
                nc.gpsimd.memset(
                    vl.rearrange('p i (h e) -> p i h e', e=65)
                    [32:33, :, :, 64:65], 1.0)
                ot = lay.tile([128, 4, NT], BF16, tag='ot', bufs=2)
                rn2 = {}

                # batched tail-token V: one matmul set for all 4 images'
                # last patch token (strided lhsT), then SBUF stage + 2 DMA
                # scatters into vl rows 32/64
                xTi = xT[:, :, 0:NT].rearrange('p c (i n) -> p c i n', n=N)
                vt_ps = pstile()
                for c in range(4):
                    nc.tensor.matmul(vt_ps[0:NIMG, :],
                                     xTi[:, c, :, N - 1],
                                     qkvw_sb[:, c, 2 * D:3 * D],
                                     start=(c == 0), stop=(c == 3))
                vt_sb = lay.tile([NIMG, D], BF16, tag='vt4', bufs=1)
                nc.vector.tensor_copy(out=vt_sb, in_=vt_ps[0:NIMG, :])
                vli = vl.rearrange('p i (h e) -> p i h e', e=65)
                for i in range(NIMG):
                    src = vt_sb[i:i + 1].rearrange('i (h e) -> i h e', e=64)
                    nc.sync.dma_start(out=vli[0:1, i, :, 0:64], in_=src)
                    nc.sync.dma_start(out=vli[32:33, i, :, 0:64], in_=src)

                def emit_bv_chunk(img, c2):
                    io = IMGOFF[img]
                    vp = pstile()
                    for c in range(4):
                        nc.tensor.matmul(
                            vp, xT[:, c, io + c2 * 128:io + (c2 + 1) * 128],
                            qkvw_sb[:, c, 2 * D:3 * D],
                            start=(c == 0), stop=(c == 3))
                    nc.vector.tensor_copy(
                        out=vimg.rearrange(
                            'p i c (h e) -> p i c h e', e=65)
                        [:, img, c2, :, 0:64],
                        in_=vp)

                def emit_bv(img):
                    for c2 in range(2):
                        emit_bv_chunk(img, c2)

                def emit_proj(t):
                    rows = _tile_rows(t)
                    pp = pstile()
                    for c in range(4):
                        nc.tensor.matmul(
                            pp[0:rows, :],
                            ot[:, c, t * 128:t * 128 + rows],
                            projw_sb[:, c, :],
                            start=(c == 0), stop=(c == 3),
                            skip_group_check=True)
                    nc.vector.tensor_add(tok[0:rows, t, :],
                                         tok[0:rows, t, :], pp[0:rows, :])
                    nc.gpsimd.tensor_add(tok[0:rows, t, :],
                                         tok[0:rows, t, :],
                                         projb_bc[0:rows, :])
                    rn2[t] = ln_stats_rstd(t)

                def stage1(img, j):
                    """Scores + tail scores + exps + diag masks for one
                    head pair. PE work here is independent of the previous
                    unit's late chain, so emitting it first keeps the PE
                    FIFO from stalling on cross-engine dependencies."""
                    io = IMGOFF[img]
                    qfc, kfc = j, 4 + j
                    h0, h1 = 2 * j, 2 * j + 1
                    sA = ps2.tile([128, 2, 512], F32, tag='stp2', bufs=2)
                    sB = ps2.tile([128, 2, 512], F32, tag='stp2', bufs=2)
                    tl = pstile()
                    for c in range(2):
                        nc.tensor.matmul(
                            sA[:, c, 0:NQ],
                            qkt[0:64, kfc, io + c * 128:io + (c + 1) * 128],
                            qkt[0:64, qfc, io:io + NQ],
                            start=True, stop=True, skip_group_check=True)
                        nc.tensor.matmul(
                            sB[:, c, 0:NQ],
                            qkt[64:128, kfc, io + c * 128:io + (c + 1) * 128],
                            qkt[64:128, qfc, io:io + NQ],
                            start=True, stop=True, skip_group_check=True)
                    nc.tensor.matmul(
                        tl[0:1, 0:NQ],
                        qkt[0:64, kfc, io + 256:io + 257],
                        qkt[0:64, qfc, io:io + NQ],
                        start=True, stop=True, skip_group_check=True)
                    nc.tensor.matmul(
                        tl[32:33, 0:NQ],
                        qkt[64:128, kfc, io + 256:io + 257],
                        qkt[64:128, qfc, io:io + NQ],
                        start=True, stop=True, skip_group_check=True)
                    ptA = lay.tile([128, 3, NQ], BF16, tag='pt', bufs=5)
                    ptB = lay.tile([128, 3, NQ], BF16, tag='pt', bufs=5)
                    nc.scalar.activation(
                        out=ptA[:, 0:2, :], in_=sA[:, :, 0:NQ],
                        func=AF.Exp, scale=temp_sb[:, h0:h0 + 1])
                    nc.scalar.activation(
                        out=ptB[:, 0:2, :], in_=sB[:, :, 0:NQ],
                        func=AF.Exp, scale=temp_sb[:, h1:h1 + 1])
                    nc.scalar.activation(
                        out=ptA[0:33, 2, :], in_=tl[0:33, 0:NQ],
                        func=AF.Exp, scale=tpair[0:33, j:j + 1])
                    nc.vector.tensor_mul(ptA[:, 0, 0:128],
                                         ptA[:, 0, 0:128], om_id)
                    nc.vector.tensor_mul(ptA[:, 1, 128:256],
                                         ptA[:, 1, 128:256], om_id)
                    nc.vector.tensor_mul(ptB[:, 0, 0:128],
                                         ptB[:, 0, 0:128], om_id)
                    nc.vector.tensor_mul(ptB[:, 1, 128:256],
                                         ptB[:, 1, 128:256], om_id)
                    nc.vector.memset(ptA[0:1, 2, 256:257], 0.0)
                    nc.vector.memset(ptA[32:33, 2, 256:257], 0.0)
                    return (img, j, ptA, ptB)

                def stage2(st):
                    """pv accumulation + softmax denominators + ot for the
                    PREVIOUS unit (emitted after the next unit's scores so
                    the PE never waits on this unit's ACT chain)."""
                    img, j, ptA, ptB = st
                    io = IMGOFF[img]
                    qfc = j
                    h0, h1 = 2 * j, 2 * j + 1
                    pv0 = pstile()
                    pv1 = pstile()
                    for c in range(2):
                        nc.tensor.matmul(
                            pv0[0:65, 0:NQ],
                            vimg[:, img, c, h0 * 65:h0 * 65 + 65],
                            ptA[:, c, :],
                            start=(c == 0), stop=False,
                            skip_group_check=True)
                        nc.tensor.matmul(
                            pv1[0:65, 0:NQ],
                            vimg[:, img, c, h1 * 65:h1 * 65 + 65],
                            ptB[:, c, :],
                            start=(c == 0), stop=False,
                            skip_group_check=True)
                    nc.tensor.matmul(
                        pv0[0:65, 0:NQ],
                        vl[0:1, img, h0 * 65:h0 * 65 + 65],
                        ptA[0:1, 2, :],
                        start=False, stop=True, skip_group_check=True)
                    nc.tensor.matmul(
                        pv1[0:65, 0:NQ],
                        vl[32:33, img, h1 * 65:h1 * 65 + 65],
                        ptA[32:33, 2, :],
                        start=False, stop=True, skip_group_check=True)
                    # 1/denom: lns at partitions 0/32, one batched exp,
                    # then stride-0 DMA broadcasts into SBUF (no PE matmul
                    # blocking the next unit's scores in the FIFO)
                    rl = lay.tile([33, NQ], F32, tag='rl', bufs=2)
                    nc.scalar.activation(out=rl[0:1, 0:NQR],
                                         in_=pv0[64:65, 0:NQR], func=AF.Ln)
                    nc.scalar.activation(out=rl[32:33, 0:NQR],
                                         in_=pv1[64:65, 0:NQR], func=AF.Ln)
                    rr = lay.tile([33, NQ], BF16, tag='rr', bufs=3)
                    nc.scalar.activation(out=rr[:, 0:NQR], in_=rl[:, 0:NQR],
                                         func=AF.Exp, scale=-1.0)
                    return (img, j, pv0, pv1, rr)

                def stage2b(st2):
                    """1/denom broadcast + ot apply, emitted a filler later
                    so the PE reaches these matmuls after the ACT chain has
                    produced rr."""
                    img, j, pv0, pv1, rr = st2
                    io = IMGOFF[img]
                    nc.tensor.matmul(pv0[64:128, 0:NQR], onesb[0:1, :],
                                     rr[0:1, 0:NQR],
                                     start=True, stop=True,
                                     skip_group_check=True)
                    nc.tensor.matmul(pv1[64:128, 0:NQR], onesb[32:33, :],
                                     rr[32:33, 0:NQR],
                                     start=True, stop=True,
                                     skip_group_check=True)
                    rts = lay.tile([128, NQ], BF16, tag='rts', bufs=3)
                    nc.vector.tensor_copy(out=rts[0:64, 0:NQR],
                                          in_=pv0[64:128, 0:NQR])
                    nc.vector.tensor_copy(out=rts[64:128, 0:NQR],
                                          in_=pv1[64:128, 0:NQR])
                    nc.vector.tensor_mul(ot[0:64, j, io:io + NQR],
                                         pv0[0:64, 0:NQR], rts[0:64, 0:NQR])
                    nc.vector.tensor_mul(ot[64:128, j, io:io + NQR],
                                         pv1[0:64, 0:NQR],
                                         rts[64:128, 0:NQR])

                emit_bv(0)
                # software-pipelined emission: stage1(k) | stage2(k-1) |
                # one dense filler per slot keeps the PE warm
                fillers = {2: ('bv', 1, 0), 3: ('bv', 1, 1),
                           5: ('proj', 0), 6: ('proj', 1),
                           7: ('bv', 2, 0), 8: ('bv', 2, 1),
                           9: ('proj', 2), 10: ('proj', 3),
                           11: ('bv', 3, 0), 12: ('bv', 3, 1),
                           13: ('proj', 4), 14: ('proj', 5)}
                units = [(img, j) for img in range(NIMG) for j in range(4)]
                prev = None
                for k, (img, j) in enumerate(units):
                    cur = stage1(img, j)
                    st2 = stage2(prev) if prev is not None else None
                    f = fillers.get(k)
                    if f is not None:
                        if f[0] == 'bv':
                            emit_bv_chunk(f[1], f[2])
                        else:
                            emit_proj(f[1])
                    if st2 is not None:
                        stage2b(st2)
                    prev = cur
                stage2b(stage2(prev))
                for t in (6, 7, 8):
                    emit_proj(t)
                return rn2

            def mlp_phase(l, xT2, ln2_rn, want_next):
                """LN2-apply + MLP(+residual) for layer l, with next layer's
                LN1 applies + B_qk fused into the MLP2 loop per tile group.
                Returns (xT_next, qkt_next, qkvw_next, ln1_rn_next)."""
                w1_sb = lay.tile([128, 4, MLP], BF16, tag='wt', bufs=3)
                nc.sync.dma_start(
                    out=w1_sb, in_=w1_d[l].rearrange('(c p) n -> p c n', p=128))
                b1_sb = lay.tile([128, 16], F32, tag='b1', bufs=2)
                nc.sync.dma_start(
                    out=b1_sb, in_=b1_d[l].rearrange('(c p) -> p c', p=128))
                b1t_sb = lay.tile([128, 64], F32, tag='b1t', bufs=2)
                nc.sync.dma_start(out=b1t_sb, in_=b1t_d[l])
                w2_sb = lay.tile([128, 16, D], BF16, tag='wt', bufs=3)
                nc.sync.dma_start(
                    out=w2_sb, in_=w2_d[l].rearrange('(c p) n -> p c n', p=128))
                b2_bc = lay.tile([128, D], BF16, tag='b2', bufs=2)
                nc.sync.dma_start(out=b2_bc,
                                  in_=b2_d[l][None, :].to_broadcast([128, D]))
                if want_next:
                    qkvw_n = lay.tile([128, 4, 3 * D], BF16, tag='wt', bufs=3)
                    nc.sync.dma_start(
                        out=qkvw_n,
                        in_=qkvw_d[l + 1].rearrange('(c p) n -> p c n', p=128))
                    qkvb_n = lay.tile([128, 8], F32, tag='qkvb', bufs=2)
                    nc.sync.dma_start(
                        out=qkvb_n,
                        in_=qkvbqk_d[l + 1].rearrange('(c p) -> p c', p=128))
                    xT_n = lay.tile([128, 4, TT], BF16, tag='xT', bufs=2)
                    qkt_n = lay.tile([128, 8, QKW], BF16, tag='qkt', bufs=2)
                else:
                    qkvw_n = qkvb_n = xT_n = qkt_n = None

                groups = ((0, 512), (512, 512), (1024, NT - 1024))
                hTs = {}

                def f_group(gi, g0, gw):
                    def emit():
                        hT = lay.tile([128, 16, gw], BF16,
                                      tag=('hT' if gw == 512 else 'hTs'),
                                      bufs=2, name=f'hT{l}_{gi}')
                        hTs[gi] = hT
                        if gw <= 4:
                            # tail tokens: all 16 hidden chunks accumulate
                            # into one bank; bias via DVE, one batched gelu
                            tp_ = pstile()
                            for hc in range(16):
                                for c in range(4):
                                    nc.tensor.matmul(
                                        tp_[:, hc * 4:hc * 4 + gw],
                                        w1_sb[:, c, hc * 128:(hc + 1) * 128],
                                        xT2[:, c, g0:g0 + gw],
                                        start=(c == 0), stop=(c == 3),
                                        skip_group_check=True)
                            nc.vector.tensor_add(tp_[:, 0:64], tp_[:, 0:64],
                                                 b1t_sb)
                            nc.scalar.activation(
                                out=hT[:, :, :],
                                in_=tp_[:, 0:64].rearrange(
                                    'p (h e) -> p h e', e=4),
                                func=AF.Gelu, scale=1.0)
                            return
                        for hc in range(16):
                            hp = pstile()
                            for c in range(4):
                                nc.tensor.matmul(
                                    hp[:, 0:gw],
                                    w1_sb[:, c, hc * 128:(hc + 1) * 128],
                                    xT2[:, c, g0:g0 + gw],
                                    start=(c == 0), stop=(c == 3))
                            nc.scalar.activation(
                                out=hT[:, hc, :], in_=hp[:, 0:gw],
                                func=AF.Gelu, bias=b1_sb[:, hc:hc + 1],
                                scale=1.0)
                    return emit

                ln_sweep(ln2_rn, xT2,
                         [f_group(gi, g0, gw)
                          for gi, (g0, gw) in enumerate(groups)])

                rn_next = {}
                mvs_next = {}
                for gi, (g0, gw) in enumerate(groups):
                    hT = hTs[gi]
                    ntr = (gw + 127) // 128
                    for tr in range(ntr):
                        t = (g0 + tr * 128) // 128
                        rows = min(128, gw - tr * 128)
                        mp = pstile()
                        for c in range(16):
                            nc.tensor.matmul(
                                mp[0:rows, :],
                                hT[:, c, tr * 128:tr * 128 + rows],
                                w2_sb[:, c, :],
                                start=(c == 0), stop=(c == 15),
                                skip_group_check=True)
                        nc.vector.tensor_add(tok[0:rows, t, :],
                                             tok[0:rows, t, :], mp[0:rows, :])
                        nc.gpsimd.tensor_add(tok[0:rows, t, :],
                                             tok[0:rows, t, :],
                                             b2_bc[0:rows, :])
                        if want_next:
                            mvs_next[t] = ln_stats(t)
                if want_next:
                    gelu_fence = hTs[2][0:1, 15, 0:2]
                    for t in range(NTILE):
                        rn_next[t] = ln_rstd(mvs_next[t], fence=gelu_fence)
                    ln_sweep(rn_next, xT_n,
                             [bqk_group(qkvw_n, qkvb_n, qkt_n, xT_n, gi)
                              for gi in range(3)])
                return xT_n, qkt_n, qkvw_n, rn_next

            # layer 0 prologue: LN1 + B_qk
            rn1 = {t: ln_stats_rstd(t) for t in range(NTILE)}
            qkvw_sb = lay.tile([128, 4, 3 * D], BF16, tag='wt', bufs=3)
            nc.sync.dma_start(
                out=qkvw_sb,
                in_=qkvw_d[0].rearrange('(c p) n -> p c n', p=128))
            qkvb_sb = lay.tile([128, 8], F32, tag='qkvb', bufs=2)
            nc.sync.dma_start(
                out=qkvb_sb,
                in_=qkvbqk_d[0].rearrange('(c p) -> p c', p=128))
            xT = lay.tile([128, 4, TT], BF16, tag='xT', bufs=2)
            qkt = lay.tile([128, 8, QKW], BF16, tag='qkt', bufs=2)
            ln_sweep(rn1, xT,
                     [bqk_group(qkvw_sb, qkvb_sb, qkt, xT, gi)
                      for gi in range(3)])
            for l in range(L):
                rn2 = attn_proj_phase(l, xT, qkt, qkvw_sb)
                xT2 = lay.tile([128, 4, TT], BF16, tag='xT', bufs=2)
                xT, qkt, qkvw_sb, rn1 = mlp_phase(l, xT2, rn2, l < L - 1)

            # ================= head =================
            hw_sb = lay.tile([128, 4, 4], F32, tag='hwsb')
            nc.sync.dma_start(out=hw_sb,
                              in_=hw_d[:, :].rearrange('(c p) n -> p c n', p=128))
            hb_sb = lay.tile([1, 4], F32, tag='hbsb')
            nc.sync.dma_start(out=hb_sb, in_=hb_d[None, :])

            cls_sb = lay.tile([NIMG, D], F32, tag='cls')
            for img in range(NIMG):
                r = IMGOFF[img]
                nc.sync.dma_start(out=cls_sb[img:img + 1, :],
                                  in_=tok[r % 128:r % 128 + 1, r // 128, :])
            # final LN on the 4 cls tokens
            stats = lay.tile([NIMG, 6], F32, tag='hstat')
            mv = lay.tile([NIMG, 2], F32, tag='hmv')
            nc.vector.bn_stats(out=stats, in_=cls_sb[0:NIMG, :])
            nc.vector.bn_aggr(out=mv, in_=stats)
            lnv = lay.tile([NIMG, 1], F32, tag='hlnv')
            nc.scalar.activation(out=lnv, in_=mv[:, 1:2], func=AF.Ln,
                                 bias=eps[0:NIMG], scale=1.0)
            rstd = lay.tile([NIMG, 1], F32, tag='hrstd')
            nc.scalar.activation(out=rstd, in_=lnv, func=AF.Exp, scale=-0.5)
            nmr = lay.tile([NIMG, 1], F32, tag='hnmr')
            nc.vector.scalar_tensor_tensor(
                out=nmr, in0=mv[:, 0:1], scalar=-1.0,
                in1=rstd, op0=ALU.mult, op1=ALU.mult)
            clsn = lay.tile([NIMG, D], F32, tag='clsn')
            nc.scalar.activation(out=clsn, in_=cls_sb[0:NIMG, :],
                                 func=AF.Identity, scale=rstd, bias=nmr)
            clsT = lay.tile([128, 4, NIMG], F32, tag='clsT')
            for c in range(4):
                tp = pstile()
                nc.tensor.transpose(tp[0:128, 0:NIMG],
                                    clsn[0:NIMG, c * 128:(c + 1) * 128],
                                    ident[0:NIMG, 0:NIMG])
                nc.vector.tensor_copy(out=clsT[:, c, :], in_=tp[0:128, 0:NIMG])
            onesf = lay.tile([1, NIMG], F32, tag='onesf')
            nc.vector.memset(onesf, 1.0)
            op = pstile()
            nc.tensor.matmul(op[0:NIMG, 0:4], onesf[0:1, 0:NIMG], hb_sb,
                             start=True, stop=False, skip_group_check=True)
            for c in range(4):
                nc.tensor.matmul(op[0:NIMG, 0:4], clsT[:, c, :],
                                 hw_sb[:, c, :],
                                 start=False, stop=(c == 3),
                                 skip_group_check=True)
            osb = lay.tile([NIMG, 4], F32, tag='osb')
            nc.vector.tensor_copy(out=osb[0:NIMG, :], in_=op[0:NIMG, 0:4])
            nc.sync.dma_start(out=out_d[:, :], in_=osb[0:NIMG, :])

    return nc


# ============================================================================
# entry point
# ============================================================================
def kernel(**inputs) -> np.ndarray:
    _install_fixups()
    from concourse.bass_utils import run_bass_kernel_spmd

    if 'nc' not in _PROGRAM_CACHE:
        _PROGRAM_CACHE['nc'] = _build_program()
    nc = _PROGRAM_CACHE['nc']

    in_maps = _host_prep(inputs)
    res = run_bass_kernel_spmd(nc, in_maps, core_ids=list(range(NCORES)))
    out = np.concatenate([np.asarray(res.results[i]['out'])
                          for i in range(NCORES)], 0)
    return out[:, :NCLS].astype(np.float32)



# revision 30
# speedup vs baseline: 1.0357x; 1.0357x over previous
"""Lensiformer forward pass on 8 Trainium2 NeuronCores.

Strategy: data-parallel over batch (32 images -> 4 per core, params
replicated, no collectives). Per core, a single fused Bass/Tile program
runs the whole network.

v3 (attention head-pair rework of the v2 engine-balance build):
  - bf16 matmul operands everywhere in the transformer (residual stream,
    LN statistics and PSUM accumulation stay fp32)
  - attention processed as head PAIRS (2j, 2j+1): the pair's score and
    tail matmuls sit at partition halves 0:64 / 64:128 of the shared fc
    columns, so they auto-derive distinct PE row groups and run
    concurrently when issued adjacently
  - no mask matmuls: the self-attention diagonal is zeroed post-exp on
    SBUF bf16 (pt *= 1-I); the per-image tail key (token 256) gets a
    1-element memset; NQ=257 (no pad column)
  - tail-token V for all 4 images via one strided-lhsT matmul set per
    layer, staged to SBUF and DMA-scattered to partitions 0/32 so the
    pair's pv-tail matmuls land on distinct row groups
  - softmax denominator via a 65th all-ones V column; 1/denom =
    exp(-ln(d)) with the pair's lns at partitions 0/32 and one batched
    exp; rank-1 broadcasts into the pv dead rows (concurrent pair)
  - software-pipelined emission: stage1(unit k) scores/exps | stage2
    (unit k-1) pv/denoms | one dense proj/B_v filler per slot | stage2b
    (unit k-1) broadcast+ot-apply, so the PE FIFO rarely waits on the
    cross-engine softmax chain
  - LN applies on DVE (tensor_scalar) instead of ACT; MLP1 tail tokens
    batched into one PSUM bank with host-prepared b1tail bias and a
    single gelu
  - LN rstd = exp(-0.5*ln(var+eps)) so LN + attention exp + copies all
    live in one ACT table; gelu is the only table switch (2/layer)
  - PSUM: 4 x 1-bank rotating tiles + 2 x 2-bank score tiles

Self-contained: includes the walrus sync-wait-limit workaround and the
axon NTFF profiling shim.
"""
import contextlib
import ctypes
import sys
import types

import numpy as np
import ml_dtypes

import concourse.bass as bass
import concourse.mybir as mybir
import concourse.tile as tile
from concourse.masks import make_identity
from concourse.vector_clock import ScopedClock

F32 = mybir.dt.float32
F32R = mybir.dt.float32r
BF16 = mybir.dt.bfloat16
AF = mybir.ActivationFunctionType
ALU = mybir.AluOpType

# ---------------- model geometry (hardcoded from the problem spec) ----------
B, IMG, PATCH = 32, 128, 8
D, H, L, MLP, NCLS = 512, 8, 8, 2048, 3
GRID = IMG // PATCH            # 16
P = GRID * GRID                # 256 patches / image
N = P + 1                      # 257 tokens / image
HD = D // H                    # 64
KC = 320                       # im2col contraction: 5 shifts * 8 * 8
NCORES = 8
NIMG = B // NCORES             # 4 images / core
TP = NIMG * P                  # 1024 patch tokens / core
NT = NIMG * N                  # 1028 transformer tokens / core
NTILE = 9                      # token tiles of 128
TT = NTILE * 128               # 1152 padded tokens
IMGOFF = [i * N for i in range(NIMG)]
NEG = -1.0e30
NQ = 257                       # score q width (real q cols only)
NQR = 257                      # real q cols
QKW = NT                       # qkt cols
BF_NP = ml_dtypes.bfloat16

_PROGRAM_CACHE = {}

# ============================================================================
# environment fixups
# ============================================================================
_fixups_done = False


def _install_fixups():
    global _fixups_done
    if _fixups_done:
        return
    _fixups_done = True
    MAXW = 1

    def _split_waits(nc, ordered):
        for bb_name, insts in ordered.items():
            new_list = []
            for inst in insts:
                si = getattr(inst, 'sync_info', None)
                eng = getattr(inst, 'engine', None)
                if (si is not None and si.on_wait and len(si.on_wait) > MAXW
                        and eng is not None
                        and type(inst).__name__.startswith('Inst')):
                    waits = list(si.on_wait)
                    inst.sync_info = mybir.SyncInfo(
                        on_wait=waits[:MAXW], on_update=list(si.on_update or []))
                    for i in range(MAXW, len(waits), MAXW):
                        new_list.append(mybir.InstNoOp(
                            name=nc.get_next_instruction_name(),
                            engine=eng, bass_nofuse=True,
                            sync_info=mybir.SyncInfo(
                                on_wait=waits[i:i + MAXW], on_update=[])))
                new_list.append(inst)
            ordered[bb_name] = new_list

    orig_lower = tile.TileContext._lower_ordered_insts

    def patched_lower(self, ordered):
        _split_waits(self.nc, ordered)
        return orig_lower(self, ordered)

    tile.TileContext._lower_ordered_insts = patched_lower

    def patched_drain_and_barrier(self, tick_clock, wait_clock):
        drain_inst = self.nc.sync.drain()
        wait_clock.add_sem_waits(
            drain_inst.ins, ScopedClock({None: tick_clock.global_clock}))
        si = drain_inst.ins.sync_info
        if si and si.on_wait and len(si.on_wait) > MAXW:
            waits = list(si.on_wait)
            drain_inst.ins.sync_info = mybir.SyncInfo(
                on_wait=waits[:MAXW], on_update=list(si.on_update or []))
            for i in range(MAXW, len(waits), MAXW):
                extra = self.nc.sync.drain()
                extra.ins.sync_info = mybir.SyncInfo(
                    on_wait=waits[i:i + MAXW], on_update=[])
        self.nc.all_engine_barrier()
        assert self.sems is not None
        popped = self.nc._tile_sem_poison_stack.pop()
        assert popped is self._sem_poison
        self.nc.clear_and_free_semaphores(list(self.sems.allocated().values()))
        self.nc.all_engine_barrier()

    tile.TileContext._drain_and_barrier = patched_drain_and_barrier

    if 'antenv.axon_hooks' not in sys.modules:
        holder = {'h': None}
        mod = types.ModuleType('antenv.axon_hooks')
        mod.set_axon_ntff_profile_hook = lambda h: holder.__setitem__('h', h)
        mod.get_axon_ntff_profile_hook = lambda: holder['h']
        sys.modules['antenv.axon_hooks'] = mod
        try:
            lib = ctypes.CDLL('/opt/axon/libaxon_pjrt.so')
            if hasattr(lib, 'axon_start_nrt_profile'):
                lib.axon_start_nrt_profile.argtypes = [
                    ctypes.POINTER(ctypes.c_int64), ctypes.c_size_t]
                lib.axon_start_nrt_profile.restype = ctypes.c_int64
                lib.axon_stop_nrt_profile.argtypes = [ctypes.c_char_p]
                lib.axon_stop_nrt_profile.restype = ctypes.c_int64

                @contextlib.contextmanager
                def _hook(output_dir, device_ids):
                    import jax
                    jax.devices()
                    if device_ids:
                        ids = (ctypes.c_int64 * len(device_ids))(*device_ids)
                        rc = lib.axon_start_nrt_profile(ids, len(device_ids))
                    else:
                        rc = lib.axon_start_nrt_profile(None, 0)
                    if rc != 0:
                        raise RuntimeError(f'axon_start_nrt_profile rc={rc}')
                    try:
                        yield
                    finally:
                        lib.axon_stop_nrt_profile(output_dir.encode())

                mod.set_axon_ntff_profile_hook(_hook)
        except OSError:
            pass


# ============================================================================
# host-side input marshaling (pure data movement + tiny param folds)
# ============================================================================
def _im2col(image):
    """(Bc,1,IMG,IMG) -> (Bc, P, 320), col order [shift, py, px]."""
    shifts = [(0, 0), (1, 1), (-1, 1), (1, -1), (-1, -1)]
    x = image[:, 0]
    cols = []
    for (sy, sx) in shifts:
        xs = np.roll(x, (sy, sx), (1, 2))
        pt = xs.reshape(-1, GRID, PATCH, GRID, PATCH).transpose(0, 1, 3, 2, 4)
        cols.append(pt.reshape(-1, P, PATCH * PATCH))
    return np.concatenate(cols, -1)


def _bf(a):
    return np.ascontiguousarray(np.asarray(a, np.float32).astype(BF_NP))


def _host_prep(inputs):
    f = lambda k: np.ascontiguousarray(np.asarray(inputs[k], np.float32))
    image = f('image')

    # conv weights -> matmul form, both tokenizers side by side
    wconv = np.concatenate(
        [f('ssw').reshape(D, KC).T, f('sow').reshape(D, KC).T], 1)  # (320,1024)
    bconv = np.concatenate([f('ssb'), f('sob')])                    # (1024,)
    gbeta = np.stack([np.concatenate([f('ssg'), f('sog')]),
                      np.concatenate([f('ssbeta'), f('sobeta')])])  # (2,1024)

    # fold LN gains/biases into the following matmuls (exact rewrite)
    ln1g, ln1b = f('ln1g'), f('ln1b')
    ln2g, ln2b = f('ln2g'), f('ln2b')
    qkvw, qkvb = f('qkvw'), f('qkvb')
    projw, projb = f('projw'), f('projb')
    w1, b1 = f('w1'), f('b1')
    qkvw_eff = ln1g[:, :, None] * qkvw
    qkvb_eff = qkvb + np.einsum('ld,ldn->ln', ln1b, qkvw)
    w1_eff = ln2g[:, :, None] * w1
    b1_eff = b1 + np.einsum('ld,ldn->ln', ln2b, w1)
    hw_eff = f('ng')[:, None] * f('hw')
    # V bias contributes exactly bv @ projw to the proj output (softmax
    # weights sum to 1 after the 1/denom divide) -> fold into projb
    bv = qkvb_eff[:, 2 * D:3 * D]                      # (L, 512)
    projb_eff = projb + np.einsum('ld,ldn->ln', bv, projw)

    # pos/cls in padded transformer layout
    pos = f('pos_embed')[0]          # (257, 512)
    cls_eff = f('cls_token')[0, 0] + pos[0]
    pospad = np.zeros((TT, D), np.float32)
    for i in range(NIMG):
        pospad[IMGOFF[i]] = cls_eff
        pospad[IMGOFF[i] + 1: IMGOFF[i] + N] = pos[1:]

    X = _im2col(image)               # (B, P, 320)

    # MLP1 tail bias, pre-broadcast for the batched tail bank:
    # b1tail[l, p, hc*4 + j] = b1_eff[l, hc*128 + p]
    b1tail = np.repeat(
        b1_eff.reshape(L, MLP // 128, 128).transpose(0, 2, 1), 4, axis=2)

    common = dict(
        wconv=_bf(wconv), bconv=_bf(bconv), gbeta=_bf(gbeta),
        fw=_bf(f('fw')), fb=_bf(f('fb')), pospad=pospad,
        qkvw=_bf(qkvw_eff),
        qkvbqk=np.ascontiguousarray(qkvb_eff[:, 0:2 * D]),
        projw=_bf(projw), projb=_bf(projb_eff), temp=f('temp'),
        w1=_bf(w1_eff), b1=np.ascontiguousarray(b1_eff),
        b1tail=np.ascontiguousarray(b1tail),
        w2=_bf(f('w2')), b2=_bf(f('b2')),
        hw=np.ascontiguousarray(
            np.concatenate([hw_eff, np.zeros((D, 1), np.float32)], 1)),
        hb=np.ascontiguousarray(
            np.concatenate([f('hb') + f('nb') @ f('hw'),
                            np.zeros(1, np.float32)])),
    )
    in_maps = []
    for c in range(NCORES):
        # token-layout im2col: col = transformer token index, cls cols zero
        xt = np.zeros((KC, NT), np.float32)
        for i in range(NIMG):
            xt[:, IMGOFF[i] + 1:IMGOFF[i] + N] = X[c * NIMG + i].T
        m = dict(common)
        m['xt'] = _bf(xt)
        in_maps.append(m)
    return in_maps


# ============================================================================
# device program
# ============================================================================
def _tile_segments(t):
    """Real-token segments of token-tile t: (row_in_tile, n, img, pos0)."""
    segs = []
    r0 = 128 * t
    for img in range(NIMG):
        lo = max(r0, IMGOFF[img])
        hi = min(r0 + 128, IMGOFF[img] + N, NT)
        if lo < hi:
            segs.append((lo - r0, hi - lo, img, lo - IMGOFF[img]))
    return segs


def _tile_rows(t):
    """Number of real token rows in token-tile t."""
    return min(128, max(0, NT - 128 * t))


def _build_program():
    nc = bass.Bass()

    din = lambda nm, sh, dt_=F32: nc.dram_tensor(nm, sh, dt_, kind='ExternalInput')
    xt_d = din('xt', [KC, NT], BF16)
    wc_d = din('wconv', [KC, 2 * D], BF16)
    bc_d = din('bconv', [2 * D], BF16)
    gb_d = din('gbeta', [2, 2 * D], BF16)
    fw_d = din('fw', [2 * D, D], BF16)
    fb_d = din('fb', [D], BF16)
    pos_d = din('pospad', [TT, D])
    qkvw_d = din('qkvw', [L, D, 3 * D], BF16)
    qkvbqk_d = din('qkvbqk', [L, 2 * D])
    projw_d = din('projw', [L, D, D], BF16)
    projb_d = din('projb', [L, D], BF16)
    temp_d = din('temp', [L, H])
    w1_d = din('w1', [L, D, MLP], BF16)
    b1_d = din('b1', [L, MLP])
    b1t_d = din('b1tail', [L, 128, 64])
    w2_d = din('w2', [L, MLP, D], BF16)
    b2_d = din('b2', [L, D], BF16)
    hw_d = din('hw', [D, 4])
    hb_d = din('hb', [4])
    out_d = nc.dram_tensor('out', [NIMG, 4], F32, kind='ExternalOutput')

    with tile.TileContext(nc) as tc, \
            nc.allow_low_precision(reason='bf16 matmul operands'):
        with contextlib.ExitStack() as ctx:
            sb = ctx.enter_context(tc.tile_pool(name='sb', bufs=1))
            ps = ctx.enter_context(tc.tile_pool(name='ps', bufs=4, space='PSUM'))
            ps2 = ctx.enter_context(tc.tile_pool(name='ps2', bufs=2, space='PSUM'))

            _psn = [0]

            def pstile(dt_=F32):
                _psn[0] += 1
                return ps.tile([128, 512], dt_, tag='ps', bufs=4,
                               name=f'ps{_psn[0]}')

            # ---------------- constants ----------------
            ident = sb.tile([128, 128], F32, tag='ident')
            make_identity(nc, ident)
            ident_bf = sb.tile([128, 128], BF16, tag='identbf')
            nc.vector.tensor_copy(out=ident_bf, in_=ident)
            # 1 - I, to zero the self-attention diagonal post-exp
            om_id = sb.tile([128, 128], BF16, tag='omid')
            nc.vector.memset(om_id, 1.0)
            nc.vector.tensor_sub(om_id, om_id, ident_bf)
            # all-ones block whose rows 0 / 32 serve as 1/denom broadcast
            # lhsT for the pair (distinct row groups -> concurrent matmuls)
            onesb = sb.tile([64, 64], BF16, tag='onesb')
            nc.vector.memset(onesb, 1.0)
            ones_row = sb.tile([1, 128], BF16, tag='ones_row')
            nc.vector.memset(ones_row, 1.0)
            eps = sb.tile([128, 1], F32, tag='eps')
            nc.vector.memset(eps, 1e-5)

            # ---------------- persistent activations ----------------
            tok = sb.tile([128, NTILE, D], F32, tag='tok')       # residual

            lay = None  # transformer pool; opened after the embed pool closes

            # ---- LN helpers (split so ACT parts can be emission-grouped) ---
            def ln_stats(t):
                stats = lay.tile([128, 6], F32, tag='lnstat', bufs=10)
                mv = lay.tile([128, 2], F32, tag='lnmv', bufs=10)
                nc.vector.bn_stats(out=stats, in_=tok[:, t, :])
                nc.vector.bn_aggr(out=mv, in_=stats)
                return mv

            def ln_rstd(mv, fence=None):
                """Ln/Exp rstd chain (the ACT-table-sensitive part). `fence`
                delays it (via a value-preserving 1-element rewrite of mv)
                so the scheduler cannot pull the Ln/Exp into a gelu stretch
                and thrash the ACT table."""
                if fence is not None:
                    nc.vector.scalar_tensor_tensor(
                        out=mv[0:1, :], in0=mv[0:1, :], scalar=1.0,
                        in1=fence, op0=ALU.mult, op1=ALU.bypass)
                lnv = lay.tile([128, 1], F32, tag='lnv', bufs=10)
                nc.scalar.activation(out=lnv, in_=mv[:, 1:2], func=AF.Ln,
                                     bias=eps, scale=1.0)
                rstd = lay.tile([128, 1], F32, tag='lnrstd', bufs=10)
                nc.scalar.activation(out=rstd, in_=lnv, func=AF.Exp,
                                     scale=-0.5)
                nmr = lay.tile([128, 1], F32, tag='lnnmr', bufs=10)
                nc.vector.scalar_tensor_tensor(
                    out=nmr, in0=mv[:, 0:1], scalar=-1.0,
                    in1=rstd, op0=ALU.mult, op1=ALU.mult)
                return rstd, nmr

            def ln_apply_tr(t, rstd, nmr, xT_dst):
                """LN apply on DVE (frees ACT) + transposes + copy."""
                xn = lay.tile([128, D], BF16, tag='xn', bufs=2)
                nc.vector.tensor_scalar(
                    out=xn, in0=tok[:, t, :], scalar1=rstd, scalar2=nmr,
                    op0=ALU.mult, op1=ALU.add)
                tp = pstile(BF16)
                for c in range(4):
                    nc.tensor.transpose(tp[:, c * 128:(c + 1) * 128],
                                        xn[:, c * 128:(c + 1) * 128], ident_bf)
                nc.vector.tensor_copy(
                    out=xT_dst[:, :, t * 128:(t + 1) * 128], in_=tp)

            def ln_stats_rstd(t, fence=None):
                return ln_rstd(ln_stats(t), fence=fence)

            # tile groups matching the 512/512/tail column groups that
            # B_qk / MLP1 consume; applies interleave with those groups
            _LN_GROUPS = ((0, 1, 2, 3), (4, 5, 6, 7), (8,))

            def ln_sweep(rn, xT_dst, group_emitters=None):
                for gi, tiles in enumerate(_LN_GROUPS):
                    for t in tiles:
                        ln_apply_tr(t, rn[t][0], rn[t][1], xT_dst)
                    if group_emitters is not None:
                        group_emitters[gi]()

            # ================= patch embed =================
            # the im2col input is already in transformer token layout (cls
            # cols zero), so conv/gate/fuse run per token tile and tokens
            # land in `tok` without a DRAM reshuffle
            with tc.tile_pool(name='emb', bufs=1) as emb:
                xt_sb = []
                for kc, k0, kn in ((0, 0, 128), (1, 128, 128), (2, 256, 64)):
                    t_ = emb.tile([kn, NT], BF16, tag=f'xt{kc}')
                    nc.sync.dma_start(out=t_, in_=xt_d[k0:k0 + kn, :])
                    xt_sb.append(t_)
                wc_sb = []
                for kc, k0, kn in ((0, 0, 128), (1, 128, 128), (2, 256, 64)):
                    t_ = emb.tile([kn, 2 * D], BF16, tag=f'wc{kc}')
                    nc.sync.dma_start(out=t_, in_=wc_d[k0:k0 + kn, :])
                    wc_sb.append(t_)
                bc_sb = emb.tile([1, 2 * D], BF16, tag='bc')
                nc.sync.dma_start(out=bc_sb, in_=bc_d[None, :])
                gb_g = emb.tile([128, 2 * D], BF16, tag='gbg')
                nc.sync.dma_start(
                    out=gb_g, in_=gb_d[0][None, :].to_broadcast([128, 2 * D]))
                gb_b = emb.tile([128, 2 * D], BF16, tag='gbb')
                nc.sync.dma_start(
                    out=gb_b, in_=gb_d[1][None, :].to_broadcast([128, 2 * D]))
                fw_sb = emb.tile([128, 8, D], BF16, tag='fwsb')
                nc.sync.dma_start(
                    out=fw_sb, in_=fw_d[:, :].rearrange('(c p) n -> p c n', p=128))
                fb_sb = emb.tile([1, D], BF16, tag='fbsb')
                nc.sync.dma_start(out=fb_sb, in_=fb_d[None, :])

                # cls token rows per token tile: (tile, row)
                cls_rows = {IMGOFF[i] // 128: IMGOFF[i] % 128
                            for i in range(NIMG)}
                nc.vector.memset(tok[:, NTILE - 1, :], 0.0)
                combs, ggs = [], []
                # sweep 1: conv + LN + gains + transposes + gate matmul
                # (ACT stays in the ln/exp/identity table)
                for t in range(NTILE):
                    rows = _tile_rows(t)
                    c0 = t * 128
                    combraw = emb.tile([128, 2 * D], F32, tag='combraw', bufs=3)
                    for nh in range(2):
                        cps = pstile()
                        nc.tensor.matmul(cps[0:rows, :], ones_row[0:1, 0:rows],
                                         bc_sb[0:1, nh * D:(nh + 1) * D],
                                         start=True, stop=False,
                                         skip_group_check=True)
                        for kc in range(3):
                            nc.tensor.matmul(
                                cps[0:rows, :], xt_sb[kc][:, c0:c0 + rows],
                                wc_sb[kc][:, nh * D:(nh + 1) * D],
                                start=False, stop=(kc == 2),
                                skip_group_check=True)
                        nc.vector.tensor_copy(
                            out=combraw[0:rows, nh * D:(nh + 1) * D],
                            in_=cps[0:rows, :])

                    comb = emb.tile([128, 2 * D], BF16, tag='comb', bufs=9)
                    for nh in range(2):
                        sl = slice(nh * D, (nh + 1) * D)
                        stats = emb.tile([128, 6], F32, tag='estat', bufs=4)
                        mv = emb.tile([128, 2], F32, tag='emv', bufs=4)
                        nc.vector.bn_stats(out=stats[0:rows],
                                           in_=combraw[0:rows, sl])
                        nc.vector.bn_aggr(out=mv[0:rows], in_=stats[0:rows])
                        lnv = emb.tile([128, 1], F32, tag='elnv', bufs=4)
                        nc.scalar.activation(out=lnv[0:rows],
                                             in_=mv[0:rows, 1:2],
                                             func=AF.Ln, bias=eps[0:rows],
                                             scale=1.0)
                        rstd = emb.tile([128, 1], F32, tag='erstd', bufs=4)
                        nc.scalar.activation(out=rstd[0:rows], in_=lnv[0:rows],
                                             func=AF.Exp, scale=-0.5)
                        nmr = emb.tile([128, 1], F32, tag='enmr', bufs=4)
                        nc.vector.scalar_tensor_tensor(
                            out=nmr[0:rows], in0=mv[0:rows, 0:1], scalar=-1.0,
                            in1=rstd[0:rows], op0=ALU.mult, op1=ALU.mult)
                        nc.scalar.activation(out=comb[0:rows, sl],
                                             in_=combraw[0:rows, sl],
                                             func=AF.Identity,
                                             scale=rstd[0:rows],
                                             bias=nmr[0:rows])
                        last_rstd = rstd
                    nc.vector.tensor_mul(comb[0:rows], comb[0:rows], gb_g[0:rows])
                    nc.vector.tensor_add(comb[0:rows], comb[0:rows], gb_b[0:rows])

                    combT = emb.tile([128, 8, 128], BF16, tag='combT', bufs=3)
                    for half in range(2):
                        tp = ps2.tile([128, 2, 512], BF16, tag='stp2', bufs=2,
                                      name=f'etp{t}_{half}')
                        for c in range(4):
                            cc = half * 4 + c
                            nc.tensor.transpose(
                                tp[:, 0, c * rows:(c + 1) * rows],
                                comb[0:rows, cc * 128:(cc + 1) * 128],
                                ident_bf[0:rows, 0:rows])
                        nc.vector.tensor_copy(
                            out=combT[:, half * 4:(half + 1) * 4, 0:rows],
                            in_=tp[:, 0, 0:4 * rows].rearrange(
                                'p (c n) -> p c n', c=4))

                    gps = pstile()
                    nc.tensor.matmul(gps[0:rows, :], ones_row[0:1, 0:rows],
                                     fb_sb, start=True, stop=False,
                                     skip_group_check=True)
                    for c in range(8):
                        nc.tensor.matmul(gps[0:rows, :], combT[:, c, 0:rows],
                                         fw_sb[:, c, :],
                                         start=False, stop=(c == 7),
                                         skip_group_check=True)
                    gg = emb.tile([128, D], F32, tag='gg', bufs=9)
                    nc.scalar.copy(out=gg[0:rows], in_=gps[0:rows, :])
                    combs.append(comb)
                    ggs.append(gg)

                # sweep 2: sigmoids, fenced behind the last embed-LN rstd
                # so the scheduler cannot interleave them with the Ln/Exp
                # chains and thrash the ACT table; then fuse into tok
                for t in range(NTILE):
                    rows = _tile_rows(t)
                    nc.vector.scalar_tensor_tensor(
                        out=ggs[t][0:1, 0:1], in0=ggs[t][0:1, 0:1],
                        scalar=1.0, in1=last_rstd[0:1, 0:1],
                        op0=ALU.mult, op1=ALU.bypass)
                    gt = emb.tile([128, D], F32, tag='gt', bufs=2)
                    nc.scalar.activation(out=gt[0:rows], in_=ggs[t][0:rows],
                                         func=AF.Sigmoid)
                    diff = emb.tile([128, D], F32, tag='diff', bufs=2)
                    nc.vector.tensor_sub(diff[0:rows], combs[t][0:rows, 0:D],
                                         combs[t][0:rows, D:2 * D])
                    nc.vector.tensor_mul(diff[0:rows], diff[0:rows],
                                         gt[0:rows])
                    nc.vector.tensor_add(diff[0:rows], diff[0:rows],
                                         combs[t][0:rows, D:2 * D])
                    postile = emb.tile([128, D], F32, tag='pos', bufs=3)
                    nc.gpsimd.dma_start(out=postile,
                                        in_=pos_d[t * 128:(t + 1) * 128, :])
                    nc.vector.tensor_add(tok[0:rows, t, :], diff[0:rows],
                                         postile[0:rows])
                    if t in cls_rows:
                        # cls rows carry conv-of-zeros junk; overwrite with
                        # cls_eff straight from pospad (DMA may address any
                        # partition, unlike the compute engines)
                        r = cls_rows[t]
                        nc.sync.dma_start(
                            out=tok[r:r + 1, t, :],
                            in_=pos_d[t * 128 + r:t * 128 + r + 1, :])

            # ================= transformer layers =================
            lay = ctx.enter_context(tc.tile_pool(name='lay', bufs=1))

            # after C(img), ot cols for these token tiles are complete
            _D_TILES = {0: (0, 1), 1: (2, 3), 2: (4, 5), 3: (6, 7, 8)}

            _BQK_GROUPS = ((0, 512), (512, 512), (1024, QKW - 1024))

            def bqk_group(qkvw_sb, qkvb_sb, qkt, xT, gi):
                g0, gw = _BQK_GROUPS[gi]

                def emit():
                    for fc in range(8):
                        qp = pstile()
                        for c in range(4):
                            nc.tensor.matmul(
                                qp[:, 0:gw],
                                qkvw_sb[:, c, fc * 128:(fc + 1) * 128],
                                xT[:, c, g0:g0 + gw],
                                start=(c == 0), stop=(c == 3))
                        nc.vector.tensor_scalar_add(
                            qkt[:, fc, g0:g0 + gw], qp[:, 0:gw],
                            qkvb_sb[:, fc:fc + 1])
                return emit

            def attn_proj_phase(l, xT, qkt, qkvw_sb, fillers=None):
                """B_v + C + proj(+residual) for layer l, head-PAIR units.

                Heads 2j/2j+1 share fc columns at partition halves 0:64 /
                64:128, so their score and tail matmuls auto-derive distinct
                PE row groups and run CONCURRENTLY when issued adjacently.
                The diagonal mask is applied post-exp on SBUF bf16 (pt *=
                1-I) instead of PSUM adds; both tail-score rows share one pv
                bank so a single ACT exp covers them; 1/denom is broadcast
                for the pair with one e2 matmul into the recycled score
                bank. Returns LN2 rstd chains per tile."""
                temp_sb = lay.tile([128, H], F32, tag='temp', bufs=2)
                nc.sync.dma_start(out=temp_sb,
                                  in_=temp_d[l][None, :].to_broadcast([128, H]))
                # per-pair tail-exp scale: temp[2j] at partition 32,
                # temp[2j+1] at partition 64 (one AP serves the batched exp)
                tpair = lay.tile([128, 4], F32, tag='tpair', bufs=2)
                tview = temp_sb.rearrange('p (j two) -> p j two', two=2)
                nc.vector.tensor_copy(out=tpair[0:1, :],
                                      in_=tview[0:1, :, 0])
                nc.vector.tensor_copy(out=tpair[32:33, :],
                                      in_=tview[32:33, :, 1])
                projw_sb = lay.tile([128, 4, D], BF16, tag='wt', bufs=3)
                nc.sync.dma_start(
                    out=projw_sb,
                    in_=projw_d[l].rearrange('(c p) n -> p c n', p=128))
                projb_bc = lay.tile([128, D], BF16, tag='projb', bufs=2)
                nc.sync.dma_start(out=projb_bc,
                                  in_=projb_d[l][None, :].to_broadcast([128, D]))

                vimg = lay.tile([128, NIMG, 2, H * 65], BF16, tag='vimg',
                                bufs=1)
                nc.gpsimd.memset(
                    vimg.rearrange('p i c (h e) -> p i c h e', e=65)
                    [:, :, :, :, 64:65], 1.0)
                # tail-token V rows, duplicated at partitions 32 (even head)
                # and 64 (odd head) so the pair's pv-tail matmuls land on
                # distinct row groups and run concurrently
                vl = lay.tile([128, NIMG, H * 65], BF16, tag='vlast', bufs=1)
                nc.gpsimd.memset(vl[0:1], 0.0)
                nc.gpsimd.memset(vl[32:33], 0.0)
                nc.gpsimd.memset(
                    vl.rearrange('p i (h e) -> p i h e', e=65)
                    [0:1, :, :, 64:65], 1.0)
                nc.gpsimd.memset(
                    vl.rearrange('p i (h e) -> p i h e', e=65)
                    [32:33, :, :, 64:65], 1.0)
                ot = lay.tile([128, 4, NT], BF16, tag='ot', bufs=2)
                rn2 = {}

                # batched tail-token V: one matmul set for all 4 images'
                # last patch token (strided lhsT), then SBUF stage + 2 DMA
                # scatters into vl rows 32/64
                xTi = xT[:, :, 0:NT].rearrange('p c (i n) -> p c i n', n=N)
                vt_ps = pstile()
                for c in range(4):
                    nc.tensor.matmul(vt_ps[0:NIMG, :],
                                     xTi[:, c, :, N - 1],
                                     qkvw_sb[:, c, 2 * D:3 * D],
                                     start=(c == 0), stop=(c == 3))
                vt_sb = lay.tile([NIMG, D], BF16, tag='vt4', bufs=1)
                nc.vector.tensor_copy(out=vt_sb, in_=vt_ps[0:NIMG, :])
                vli = vl.rearrange('p i (h e) -> p i h e', e=65)
                for i in range(NIMG):
                    src = vt_sb[i:i + 1].rearrange('i (h e) -> i h e', e=64)
                    nc.sync.dma_start(out=vli[0:1, i, :, 0:64], in_=src)
                    nc.sync.dma_start(out=vli[32:33, i, :, 0:64], in_=src)

                def emit_bv_chunk(img, c2):
                    io = IMGOFF[img]
                    vp = pstile()
                    for c in range(4):
                        nc.tensor.matmul(
                            vp, xT[:, c, io + c2 * 128:io + (c2 + 1) * 128],
                            qkvw_sb[:, c, 2 * D:3 * D],
                            start=(c == 0), stop=(c == 3))
                    nc.vector.tensor_copy(
                        out=vimg.rearrange(
                            'p i c (h e) -> p i c h e', e=65)
                        [:, img, c2, :, 0:64],
                        in_=vp)

                def emit_bv(img):
                    for c2 in range(2):
                        emit_bv_chunk(img, c2)

                def emit_proj(t):
                    rows = _tile_rows(t)
                    pp = pstile()
                    for c in range(4):
                        nc.tensor.matmul(
                            pp[0:rows, :],
                            ot[:, c, t * 128:t * 128 + rows],
                            projw_sb[:, c, :],
                            start=(c == 0), stop=(c == 3),
                            skip_group_check=True)
                    nc.vector.tensor_add(tok[0:rows, t, :],
                                         tok[0:rows, t, :], pp[0:rows, :])
                    nc.gpsimd.tensor_add(tok[0:rows, t, :],
                                         tok[0:rows, t, :],
                                         projb_bc[0:rows, :])
                    rn2[t] = ln_stats_rstd(t)

                def stage1(img, j):
                    """Scores + tail scores + exps + diag masks for one
                    head pair. PE work here is independent of the previous
                    unit's late chain, so emitting it first keeps the PE
                    FIFO from stalling on cross-engine dependencies."""
                    io = IMGOFF[img]
                    qfc, kfc = j, 4 + j
                    h0, h1 = 2 * j, 2 * j + 1
                    sA = ps2.tile([128, 2, 512], F32, tag='stp2', bufs=2)
                    sB = ps2.tile([128, 2, 512], F32, tag='stp2', bufs=2)
                    tl = pstile()
                    for c in range(2):
                        nc.tensor.matmul(
                            sA[:, c, 0:NQ],
                            qkt[0:64, kfc, io + c * 128:io + (c + 1) * 128],
                            qkt[0:64, qfc, io:io + NQ],
                            start=True, stop=True, skip_group_check=True)
                        nc.tensor.matmul(
                            sB[:, c, 0:NQ],
                            qkt[64:128, kfc, io + c * 128:io + (c + 1) * 128],
                            qkt[64:128, qfc, io:io + NQ],
                            start=True, stop=True, skip_group_check=True)
                    nc.tensor.matmul(
                        tl[0:1, 0:NQ],
                        qkt[0:64, kfc, io + 256:io + 257],
                        qkt[0:64, qfc, io:io + NQ],
                        start=True, stop=True, skip_group_check=True)
                    nc.tensor.matmul(
                        tl[32:33, 0:NQ],
                        qkt[64:128, kfc, io + 256:io + 257],
                        qkt[64:128, qfc, io:io + NQ],
                        start=True, stop=True, skip_group_check=True)
                    ptA = lay.tile([128, 3, NQ], BF16, tag='pt', bufs=4)
                    ptB = lay.tile([128, 3, NQ], BF16, tag='pt', bufs=4)
                    # per-chunk exps: chunk 0 of both heads exps (and is
                    # diag-masked) while chunk 1 is still on ACT, so the
                    # next slot's pv c0 matmuls start ~an exp earlier
                    nc.scalar.activation(
                        out=ptA[:, 0, :], in_=sA[:, 0, 0:NQ],
                        func=AF.Exp, scale=temp_sb[:, h0:h0 + 1])
                    nc.scalar.activation(
                        out=ptB[:, 0, :], in_=sB[:, 0, 0:NQ],
                        func=AF.Exp, scale=temp_sb[:, h1:h1 + 1])
                    nc.vector.tensor_mul(ptA[:, 0, 0:128],
                                         ptA[:, 0, 0:128], om_id)
                    nc.vector.tensor_mul(ptB[:, 0, 0:128],
                                         ptB[:, 0, 0:128], om_id)
                    nc.scalar.activation(
                        out=ptA[:, 1, :], in_=sA[:, 1, 0:NQ],
                        func=AF.Exp, scale=temp_sb[:, h0:h0 + 1])
                    nc.scalar.activation(
                        out=ptB[:, 1, :], in_=sB[:, 1, 0:NQ],
                        func=AF.Exp, scale=temp_sb[:, h1:h1 + 1])
                    nc.scalar.activation(
                        out=ptA[0:33, 2, :], in_=tl[0:33, 0:NQ],
                        func=AF.Exp, scale=tpair[0:33, j:j + 1])
                    nc.vector.tensor_mul(ptA[:, 1, 128:256],
                                         ptA[:, 1, 128:256], om_id)
                    nc.vector.tensor_mul(ptB[:, 1, 128:256],
                                         ptB[:, 1, 128:256], om_id)
                    nc.vector.memset(ptA[0:1, 2, 256:257], 0.0)
                    nc.vector.memset(ptA[32:33, 2, 256:257], 0.0)
                    return (img, j, ptA, ptB)

                def stage2(st):
                    """pv accumulation + softmax denominators + ot for the
                    PREVIOUS unit (emitted after the next unit's scores so
                    the PE never waits on this unit's ACT chain)."""
                    img, j, ptA, ptB = st
                    io = IMGOFF[img]
                    qfc = j
                    h0, h1 = 2 * j, 2 * j + 1
                    pv0 = pstile()
                    pv1 = pstile()
                    for c in range(2):
                        nc.tensor.matmul(
                            pv0[0:65, 0:NQ],
                            vimg[:, img, c, h0 * 65:h0 * 65 + 65],
                            ptA[:, c, :],
                            start=(c == 0), stop=False,
                            skip_group_check=True)
                        nc.tensor.matmul(
                            pv1[0:65, 0:NQ],
                            vimg[:, img, c, h1 * 65:h1 * 65 + 65],
                            ptB[:, c, :],
                            start=(c == 0), stop=False,
                            skip_group_check=True)
                    nc.tensor.matmul(
                        pv0[0:65, 0:NQ],
                        vl[0:1, img, h0 * 65:h0 * 65 + 65],
                        ptA[0:1, 2, :],
                        start=False, stop=True, skip_group_check=True)
                    nc.tensor.matmul(
                        pv1[0:65, 0:NQ],
                        vl[32:33, img, h1 * 65:h1 * 65 + 65],
                        ptA[32:33, 2, :],
                        start=False, stop=True, skip_group_check=True)
                    # 1/denom: lns at partitions 0/32, one batched exp,
                    # then stride-0 DMA broadcasts into SBUF (no PE matmul
                    # blocking the next unit's scores in the FIFO)
                    rl = lay.tile([33, NQ], F32, tag='rl', bufs=2)
                    nc.scalar.activation(out=rl[0:1, 0:NQR],
                                         in_=pv0[64:65, 0:NQR], func=AF.Ln)
                    nc.scalar.activation(out=rl[32:33, 0:NQR],
                                         in_=pv1[64:65, 0:NQR], func=AF.Ln)
                    rr = lay.tile([33, NQ], BF16, tag='rr', bufs=2)
                    nc.scalar.activation(out=rr[:, 0:NQR], in_=rl[:, 0:NQR],
                                         func=AF.Exp, scale=-1.0)
                    return (img, j, pv0, pv1, rr)

                def stage2b(st2):
                    """1/denom broadcast + ot apply, emitted a filler later
                    so the PE reaches these matmuls after the ACT chain has
                    produced rr."""
                    img, j, pv0, pv1, rr = st2
                    io = IMGOFF[img]
                    nc.tensor.matmul(pv0[64:128, 0:NQR], onesb[0:1, :],
                                     rr[0:1, 0:NQR],
                                     start=True, stop=True,
                                     skip_group_check=True)
                    nc.tensor.matmul(pv1[64:128, 0:NQR], onesb[32:33, :],
                                     rr[32:33, 0:NQR],
                                     start=True, stop=True,
                                     skip_group_check=True)
                    rts = lay.tile([128, NQ], BF16, tag='rts', bufs=2)
                    nc.vector.tensor_copy(out=rts[0:64, 0:NQR],
                                          in_=pv0[64:128, 0:NQR])
                    nc.vector.tensor_copy(out=rts[64:128, 0:NQR],
                                          in_=pv1[64:128, 0:NQR])
                    nc.vector.tensor_mul(ot[0:64, j, io:io + NQR],
                                         pv0[0:64, 0:NQR], rts[0:64, 0:NQR])
                    nc.vector.tensor_mul(ot[64:128, j, io:io + NQR],
                                         pv1[0:64, 0:NQR],
                                         rts[64:128, 0:NQR])

                emit_bv(0)
                # software-pipelined emission: stage1(k) | stage2(k-1) |
                # one dense filler per slot keeps the PE warm
                fillers = {2: ('bv', 1, 0), 3: ('bv', 1, 1),
                           5: ('proj', 0), 6: ('proj', 1),
                           7: ('bv', 2, 0), 8: ('bv', 2, 1),
                           9: ('proj', 2), 10: ('proj', 3),
                           11: ('bv', 3, 0), 12: ('bv', 3, 1),
                           13: ('proj', 4), 14: ('proj', 5)}
                units = [(img, j) for img in range(NIMG) for j in range(4)]
                prev = None
                for k, (img, j) in enumerate(units):
                    cur = stage1(img, j)
                    st2 = stage2(prev) if prev is not None else None
                    f = fillers.get(k)
                    if f is not None:
                        if f[0] == 'bv':
                            emit_bv_chunk(f[1], f[2])
                        else:
                            emit_proj(f[1])
                    if st2 is not None:
                        stage2b(st2)
                    prev = cur
                stage2b(stage2(prev))
                for t in (6, 7, 8):
                    emit_proj(t)
                return rn2

            def mlp_phase(l, xT2, ln2_rn, want_next):
                """LN2-apply + MLP(+residual) for layer l, with next layer's
                LN1 applies + B_qk fused into the MLP2 loop per tile group.
                Returns (xT_next, qkt_next, qkvw_next, ln1_rn_next)."""
                w1_sb = lay.tile([128, 4, MLP], BF16, tag='wt', bufs=3)
                nc.sync.dma_start(
                    out=w1_sb, in_=w1_d[l].rearrange('(c p) n -> p c n', p=128))
                b1_sb = lay.tile([128, 16], F32, tag='b1', bufs=2)
                nc.sync.dma_start(
                    out=b1_sb, in_=b1_d[l].rearrange('(c p) -> p c', p=128))
                b1t_sb = lay.tile([128, 64], F32, tag='b1t', bufs=2)
                nc.sync.dma_start(out=b1t_sb, in_=b1t_d[l])
                w2_sb = lay.tile([128, 16, D], BF16, tag='wt', bufs=3)
                nc.sync.dma_start(
                    out=w2_sb, in_=w2_d[l].rearrange('(c p) n -> p c n', p=128))
                b2_sb = lay.tile([1, D], BF16, tag='b2', bufs=2)
                nc.sync.dma_start(out=b2_sb, in_=b2_d[l][None, :])
                if want_next:
                    qkvw_n = lay.tile([128, 4, 3 * D], BF16, tag='wt', bufs=3)
                    nc.sync.dma_start(
                        out=qkvw_n,
                        in_=qkvw_d[l + 1].rearrange('(c p) n -> p c n', p=128))
                    qkvb_n = lay.tile([128, 8], F32, tag='qkvb', bufs=2)
                    nc.sync.dma_start(
                        out=qkvb_n,
                        in_=qkvbqk_d[l + 1].rearrange('(c p) -> p c', p=128))
                    xT_n = lay.tile([128, 4, TT], BF16, tag='xT', bufs=2)
                    qkt_n = lay.tile([128, 8, QKW], BF16, tag='qkt', bufs=2)
                else:
                    qkvw_n = qkvb_n = xT_n = qkt_n = None

                groups = ((0, 512), (512, 512), (1024, NT - 1024))
                hTs = {}

                def f_group(gi, g0, gw):
                    def emit():
                        hT = lay.tile([128, 16, gw], BF16,
                                      tag=('hT' if gw == 512 else 'hTs'),
                                      bufs=2, name=f'hT{l}_{gi}')
                        hTs[gi] = hT
                        if gw <= 4:
                            # tail tokens: all 16 hidden chunks accumulate
                            # into one bank; bias via DVE, one batched gelu
                            tp_ = pstile()
                            for hc in range(16):
                                for c in range(4):
                                    nc.tensor.matmul(
                                        tp_[:, hc * 4:hc * 4 + gw],
                                        w1_sb[:, c, hc * 128:(hc + 1) * 128],
                                        xT2[:, c, g0:g0 + gw],
                                        start=(c == 0), stop=(c == 3),
                                        skip_group_check=True)
                            nc.vector.tensor_add(tp_[:, 0:64], tp_[:, 0:64],
                                                 b1t_sb)
                            nc.scalar.activation(
                                out=hT[:, :, :],
                                in_=tp_[:, 0:64].rearrange(
                                    'p (h e) -> p h e', e=4),
                                func=AF.Gelu, scale=1.0)
                            return
                        for hc in range(16):
                            hp = pstile()
                            for c in range(4):
                                nc.tensor.matmul(
                                    hp[:, 0:gw],
                                    w1_sb[:, c, hc * 128:(hc + 1) * 128],
                                    xT2[:, c, g0:g0 + gw],
                                    start=(c == 0), stop=(c == 3))
                            nc.scalar.activation(
                                out=hT[:, hc, :], in_=hp[:, 0:gw],
                                func=AF.Gelu, bias=b1_sb[:, hc:hc + 1],
                                scale=1.0)
                    return emit

                ln_sweep(ln2_rn, xT2,
                         [f_group(gi, g0, gw)
                          for gi, (g0, gw) in enumerate(groups)])

                rn_next = {}
                mvs_next = {}
                for gi, (g0, gw) in enumerate(groups):
                    hT = hTs[gi]
                    ntr = (gw + 127) // 128
                    for tr in range(ntr):
                        t = (g0 + tr * 128) // 128
                        rows = min(128, gw - tr * 128)
                        mp = pstile()
                        nc.tensor.matmul(mp[0:rows, :], ones_row[0:1, 0:rows],
                                         b2_sb, start=True, stop=False,
                                         skip_group_check=True)
                        for c in range(16):
                            nc.tensor.matmul(
                                mp[0:rows, :],
                                hT[:, c, tr * 128:tr * 128 + rows],
                                w2_sb[:, c, :],
                                start=False, stop=(c == 15),
                                skip_group_check=True)
                        nc.vector.tensor_add(tok[0:rows, t, :],
                                             tok[0:rows, t, :], mp[0:rows, :])
                        if want_next:
                            mvs_next[t] = ln_stats(t)
                if want_next:
                    gelu_fence = hTs[2][0:1, 15, 0:2]
                    for t in range(NTILE):
                        rn_next[t] = ln_rstd(mvs_next[t], fence=gelu_fence)
                    ln_sweep(rn_next, xT_n,
                             [bqk_group(qkvw_n, qkvb_n, qkt_n, xT_n, gi)
                              for gi in range(3)])
                return xT_n, qkt_n, qkvw_n, rn_next

            # layer 0 prologue: LN1 + B_qk
            rn1 = {t: ln_stats_rstd(t) for t in range(NTILE)}
            qkvw_sb = lay.tile([128, 4, 3 * D], BF16, tag='wt', bufs=3)
            nc.sync.dma_start(
                out=qkvw_sb,
                in_=qkvw_d[0].rearrange('(c p) n -> p c n', p=128))
            qkvb_sb = lay.tile([128, 8], F32, tag='qkvb', bufs=2)
            nc.sync.dma_start(
                out=qkvb_sb,
                in_=qkvbqk_d[0].rearrange('(c p) -> p c', p=128))
            xT = lay.tile([128, 4, TT], BF16, tag='xT', bufs=2)
            qkt = lay.tile([128, 8, QKW], BF16, tag='qkt', bufs=2)
            ln_sweep(rn1, xT,
                     [bqk_group(qkvw_sb, qkvb_sb, qkt, xT, gi)
                      for gi in range(3)])
            for l in range(L):
                rn2 = attn_proj_phase(l, xT, qkt, qkvw_sb)
                xT2 = lay.tile([128, 4, TT], BF16, tag='xT', bufs=2)
                xT, qkt, qkvw_sb, rn1 = mlp_phase(l, xT2, rn2, l < L - 1)

            # ================= head =================
            hw_sb = lay.tile([128, 4, 4], F32, tag='hwsb')
            nc.sync.dma_start(out=hw_sb,
                              in_=hw_d[:, :].rearrange('(c p) n -> p c n', p=128))
            hb_sb = lay.tile([1, 4], F32, tag='hbsb')
            nc.sync.dma_start(out=hb_sb, in_=hb_d[None, :])

            cls_sb = lay.tile([NIMG, D], F32, tag='cls')
            for img in range(NIMG):
                r = IMGOFF[img]
                nc.sync.dma_start(out=cls_sb[img:img + 1, :],
                                  in_=tok[r % 128:r % 128 + 1, r // 128, :])
            # final LN on the 4 cls tokens
            stats = lay.tile([NIMG, 6], F32, tag='hstat')
            mv = lay.tile([NIMG, 2], F32, tag='hmv')
            nc.vector.bn_stats(out=stats, in_=cls_sb[0:NIMG, :])
            nc.vector.bn_aggr(out=mv, in_=stats)
            lnv = lay.tile([NIMG, 1], F32, tag='hlnv')
            nc.scalar.activation(out=lnv, in_=mv[:, 1:2], func=AF.Ln,
                                 bias=eps[0:NIMG], scale=1.0)
            rstd = lay.tile([NIMG, 1], F32, tag='hrstd')
            nc.scalar.activation(out=rstd, in_=lnv, func=AF.Exp, scale=-0.5)
            nmr = lay.tile([NIMG, 1], F32, tag='hnmr')
            nc.vector.scalar_tensor_tensor(
                out=nmr, in0=mv[:, 0:1], scalar=-1.0,
                in1=rstd, op0=ALU.mult, op1=ALU.mult)
            clsn = lay.tile([NIMG, D], F32, tag='clsn')
            nc.scalar.activation(out=clsn, in_=cls_sb[0:NIMG, :],
                                 func=AF.Identity, scale=rstd, bias=nmr)
            clsT = lay.tile([128, 4, NIMG], F32, tag='clsT')
            for c in range(4):
                tp = pstile()
                nc.tensor.transpose(tp[0:128, 0:NIMG],
                                    clsn[0:NIMG, c * 128:(c + 1) * 128],
                                    ident[0:NIMG, 0:NIMG])
                nc.vector.tensor_copy(out=clsT[:, c, :], in_=tp[0:128, 0:NIMG])
            onesf = lay.tile([1, NIMG], F32, tag='onesf')
            nc.vector.memset(onesf, 1.0)
            op = pstile()
            nc.tensor.matmul(op[0:NIMG, 0:4], onesf[0:1, 0:NIMG], hb_sb,
                             start=True, stop=False, skip_group_check=True)
            for c in range(4):
                nc.tensor.matmul(op[0:NIMG, 0:4], clsT[:, c, :],
                                 hw_sb[:, c, :],
                                 start=False, stop=(c == 3),
                                 skip_group_check=True)
            osb = lay.tile([NIMG, 4], F32, tag='osb')
            nc.vector.tensor_copy(out=osb[0:NIMG, :], in_=op[0:NIMG, 0:4])
            nc.sync.dma_start(out=out_d[:, :], in_=osb[0:NIMG, :])

    return nc


# ============================================================================
# entry point
# ============================================================================
def kernel(**inputs) -> np.ndarray:
    _install_fixups()
    from concourse.bass_utils import run_bass_kernel_spmd

    if 'nc' not in _PROGRAM_CACHE:
        _PROGRAM_CACHE['nc'] = _build_program()
    nc = _PROGRAM_CACHE['nc']

    in_maps = _host_prep(inputs)
    res = run_bass_kernel_spmd(nc, in_maps, core_ids=list(range(NCORES)))
    out = np.concatenate([np.asarray(res.results[i]['out'])
                          for i in range(NCORES)], 0)
    return out[:, :NCLS].astype(np.float32)

